# revision 1
# baseline (speedup 1.0000x reference)
"""Trainium2 Bass kernel for nn_DifferentiableRiskBudgeting.

Solves, per batch sample b:
    min_w  w' S_b w - beta_b' w + lam1*||w||_1 + lam2*||w - w_prev||^2
    s.t.   sum w = 1, 0 <= w <= MAX_W
then clamps + renormalizes — matching the reference's converged
projected-gradient solution (the QP is strongly convex so the fixed
point is unique; we reach it with FISTA + a warm-started Newton
projection instead of the reference's 250 plain PGD iterations with a
30-step bisection per projection).

Sharding: pure data parallel, batch 512 = 64 samples per core on 8
cores. Each core keeps its sigma shard resident in SBUF (16.8MB) and
runs, per sample:
  - 10 power iterations (sum-of-squares normalization; the Rayleigh
    quotient at the end is scale-free) for lambda_max
  - FISTA (60 iters), per-sample step 1/L, L = 1.15*(2*lmax + 2*lam2)
  - capped-simplex projection via damped Newton on the piecewise-linear
    sum equation (slope = count+1, fused as (c1+1)-c2 so it is always
    >= 1), warm-started from the previous iteration's tau (validated:
    worst projection error 1.5e-8 over the whole trajectory)

Matvec mapping: per (sample, j-half, i-half) one PE matmul with the
sigma chunk [128,128] as the stationary operand (sigma is symmetric so
row-major storage doubles as the transposed operand) and the sample's
z-column [128,1] as the moving operand, accumulating column-major
y[i, sample] in PSUM (output base partition must be 0). DVE 32x32
block transposes read PSUM directly and produce the sample-major
y-copy the projection wants. Samples run in two groups of 32 so the
DVE/ACT projection of one group overlaps the PE matvec of the other.

Raw bass (no Tile): this container's walrus build only allows ~2 sync
commands per instruction, which Tile's scheduler exceeds at every
cross-engine join. With explicit semaphores every wait is a standalone
single-wait instruction; all semaphore values are static because the
schedule is fully unrolled. Same-engine dependent ops also need a
producer-inc + consumer-wait pair (engine pipelines do not interlock),
with ordering transitive through any later same-engine inc.
"""

import math
import numpy as np
from contextlib import ExitStack

import concourse.bass as bass
from concourse import mybir
from concourse.bass_utils import run_bass_kernel_spmd

F32 = mybir.dt.float32
ALU = mybir.AluOpType
ACTF = mybir.ActivationFunctionType

B, P = 512, 256
N_CORES = 8
NB = B // N_CORES            # samples per core
HALF = P // 128              # sigma row-halves (2)
GB = 32                      # pipeline group size
MAX_W = 0.1
EPS = 1e-8

NPOW = 10                    # power iterations
T_FISTA = 60                 # FISTA iterations
NEWTON = 5                   # Newton steps per projection
NEWTON0 = 8                  # first projection (cold tau)
SAFETY = 1.15                # L overestimation factor
SIG_DMA_BATCH = 4            # samples per sigma DMA

# set by the test harness; ignored by graders
TRACE = False
LAST_RESULT = None


def _emit(ctx, nc, sigma_d, beta_d, wprev_d, out_d, lam1, lam2, nb, npow, T):
    gb = min(GB, nb)
    ngrp = nb // gb
    assert ngrp * gb == nb and gb % 32 == 0

    def sbuf(name, shape):
        return ctx.enter_context(nc.sbuf_tensor(name, shape, F32))

    def psum(name):
        # full-bank tensors so PE writes and DVE reads of different
        # buffers can never share a PSUM bank (fatal on HW)
        return ctx.enter_context(nc.psum_tensor(name, [128, 512], F32))

    sems = {e: ctx.enter_context(nc.semaphore(f"s_{e}"))
            for e in ("pe", "dve", "act", "pool", "dma_bw", "dma_out")}
    for g in range(ngrp):
        sems[f"dma_sig{g}"] = ctx.enter_context(nc.semaphore(f"s_dma_sig{g}"))
    ENG = {"pe": nc.tensor, "dve": nc.vector, "act": nc.scalar,
           "pool": nc.gpsimd, "sync": nc.sync}
    ctr = {e: 0 for e in sems}
    last_wait = {e: {} for e in list(ENG)}

    def inc(ename, inst, n=1):
        ctr[ename] += n
        inst.then_inc(sems[ename], n)
        return ctr[ename]

    def wait(consumer, producer, value):
        if value is None or value <= 0:
            return
        lw = last_wait[consumer]
        if lw.get(producer, 0) >= value:
            return
        ENG[consumer].wait_ge(sems[producer], value)
        lw[producer] = value

    def dchain(inst):
        """Close a same-engine DVE dependency: inc on the producer, wait
        immediately after (ordering is transitive through this inc)."""
        t = inc("dve", inst)
        wait("dve", "dve", t)
        return t

    # ---------------- tensors
    ident = sbuf("ident", [128, 128])
    nbatch = SIG_DMA_BATCH
    nk = (nb + nbatch - 1) // nbatch
    sig = [sbuf(f"sig{k}", [128, nbatch * HALF * P]) for k in range(nk)]

    def sig_ap(b, hj, hi):
        k, m = divmod(b, nbatch)
        c0 = (m * HALF + hj) * P + hi * 128
        return sig[k][:, c0:c0 + 128]

    def gt(name, shape):
        return [sbuf(f"{name}{g}", shape) for g in range(ngrp)]

    z = gt("z", [gb, P])
    wA = gt("wA", [gb, P])
    wB = gt("wB", [gb, P])
    fv = gt("fv", [gb, P])
    beta_g = gt("beta", [gb, P])
    wprev_g = gt("wprev", [gb, P])
    v = gt("v", [gb, P])
    vc = gt("vc", [gb, P])
    t1 = gt("t1", [gb, P])
    dw = gt("dw", [gb, P])
    ysm = gt("ysm", [gb, P])
    outt = gt("outt", [gb, P])
    sa = gt("sa", [gb, P])       # ACT dummy outs
    sb = gt("sb", [gb, P])
    sc = gt("sc", [gb, P])       # DVE dummy outs
    sd = gt("sd", [gb, P])
    zT = [[sbuf(f"zT{g}_{h}", [128, gb]) for h in range(HALF)]
          for g in range(ngrp)]
    tiny_names = ("tau tneg tcneg s1 s2 c1 c2 phi cnt rc stp m2a dv ev "
                  "th m2 rcp num den rden lmax Lt sq onem onep rop ssum rs")
    TN = {}
    for name in tiny_names.split():
        TN[name] = gt(name, [gb, 1])

    ypsum = [[psum(f"y{g}_{p}") for p in range(2)] for g in range(ngrp)]
    ptpsum = [psum(f"pt{h}") for h in range(HALF)]

    # ---------------- events
    E_zready = {}        # g -> dve tick: z[g] ready for transpose
    E_ysm_done = {}      # (t, g) -> dve tick: y psum buffer consumed
    E_pt_free = 0        # act tick: previous pt banks consumed
    E_out_dve = {}

    # ---------------- preamble
    mz = nc.vector.memset(ident[:], 0.0)
    E_identz = inc("dve", mz)
    wait("pool", "dve", E_identz)
    af = nc.gpsimd.affine_select(
        out=ident[:], in_=ident[:], compare_op=ALU.not_equal, fill=1.0,
        base=0, pattern=[[-1, 128]], channel_multiplier=1)
    E_ident = inc("pool", af)

    kb_per_g = nk // ngrp
    for k in range(nk):
        kn = min(nbatch, nb - k * nbatch)
        srca = sigma_d[k * nbatch:k * nbatch + kn].rearrange(
            "b (h p) j -> p b h j", p=128)
        dst = sig[k][:].rearrange("p (b h j) -> p b h j", b=kn, h=HALF)
        d = nc.sync.dma_start(out=dst, in_=srca)
        d.then_inc(sems[f"dma_sig{k // kb_per_g}"], 16)
    E_sig_g = {g: 16 * kb_per_g for g in range(ngrp)}
    for g in range(ngrp):
        g0 = g * gb
        d = nc.sync.dma_start(out=beta_g[g][:], in_=beta_d[g0:g0 + gb, :])
        d.then_inc(sems["dma_bw"], 16)
        d = nc.sync.dma_start(out=wprev_g[g][:], in_=wprev_d[g0:g0 + gb, :])
        d.then_inc(sems["dma_bw"], 16)
    E_bw = 32 * ngrp

    for g in range(ngrp):
        nc.vector.memset(z[g][:], 1.0 / math.sqrt(P))
        nc.vector.memset(wA[g][:], 1.0 / P)
        m = nc.vector.memset(TN["tau"][g][:], 0.0)
        E_zready[g] = inc("dve", m)

    # ---------------- helpers
    def pe_transpose_and_matvec(t, g):
        nonlocal E_pt_free
        # transposes: z[g] -> zT (via pt psum banks), then the matvec
        wait("pe", "dve", E_zready[g])
        wait("pe", "act", E_pt_free)
        wait("pe", "pool", E_ident)
        for h in range(HALF):
            tr = nc.tensor.transpose(ptpsum[h][:, 0:gb],
                                     z[g][:, h * 128:(h + 1) * 128],
                                     ident[:gb, :gb])
            if h == HALF - 1:
                E_T = inc("pe", tr)
        # ACT copies pt -> zT
        wait("act", "pe", E_T)
        for h in range(HALF):
            cp = nc.scalar.copy(zT[g][h][:, :], ptpsum[h][:, 0:gb])
            if h == HALF - 1:
                E_zT = inc("act", cp)
        E_pt_free = E_zT
        # matvec into y psum buffer t%2
        yp = ypsum[g][t % 2]
        wait("pe", "act", E_zT)
        wait("pe", "dve", E_ysm_done.get((t - 2, g), 0))
        if t == 0:
            wait("pe", f"dma_sig{g}", E_sig_g[g])
        g0 = g * gb
        for bb in range(gb):
            b = g0 + bb
            for hi in range(HALF):
                for hj in range(HALF):
                    mm = nc.tensor.matmul(
                        yp[:, hi * gb + bb:hi * gb + bb + 1],
                        sig_ap(b, hj, hi),
                        zT[g][hj][:, bb:bb + 1],
                        start=(hj == 0), stop=(hj == HALF - 1))
        E_M = inc("pe", mm)
        return E_M

    def dve_ysm(t, g, E_M):
        # sample-major copy of y straight out of PSUM via 32x32 blocks
        yp = ypsum[g][t % 2]
        wait("dve", "pe", E_M)
        for hi in range(HALF):
            for q in range(4):
                tr = nc.vector.transpose(
                    ysm[g][0:32, hi * 128 + q * 32:hi * 128 + (q + 1) * 32],
                    yp[q * 32:(q + 1) * 32, hi * gb:hi * gb + 32])
        dchain(tr)

    # ---------------- power iterations
    for t in range(npow):
        for g in range(ngrp):
            E_M = pe_transpose_and_matvec(t, g)
            dve_ysm(t, g, E_M)
            i = nc.vector.scalar_tensor_tensor(sc[g][:], ysm[g][:], 1.0,
                                               ysm[g][:], ALU.mult, ALU.mult,
                                               accum_out=TN["m2"][g][:])
            dchain(i)
            i = nc.vector.reciprocal(TN["rcp"][g][:], TN["m2"][g][:])
            dchain(i)
            zi = nc.vector.tensor_scalar(z[g][:], ysm[g][:], TN["rcp"][g][:],
                                         None, ALU.mult)
            E_zready[g] = inc("dve", zi)

    # ---------------- Rayleigh quotient -> step sizes, FISTA coefficients
    t_ray = npow
    for g in range(ngrp):
        E_M = pe_transpose_and_matvec(t_ray, g)
        dve_ysm(t_ray, g, E_M)
        nc.vector.scalar_tensor_tensor(sc[g][:], z[g][:], 1.0, ysm[g][:],
                                       ALU.mult, ALU.mult,
                                       accum_out=TN["num"][g][:])
        i = nc.vector.scalar_tensor_tensor(sd[g][:], z[g][:], 1.0, z[g][:],
                                           ALU.mult, ALU.mult,
                                           accum_out=TN["den"][g][:])
        dchain(i)
        i = nc.vector.tensor_scalar(TN["den"][g][:], TN["den"][g][:], EPS,
                                    None, ALU.add)
        dchain(i)
        i = nc.vector.reciprocal(TN["rden"][g][:], TN["den"][g][:])
        dchain(i)
        i = nc.vector.tensor_tensor(TN["lmax"][g][:], TN["num"][g][:],
                                    TN["rden"][g][:], ALU.mult)
        dchain(i)
        i = nc.vector.tensor_scalar(TN["Lt"][g][:], TN["lmax"][g][:],
                                    2.0 * SAFETY, SAFETY * 2.0 * lam2,
                                    ALU.mult, ALU.add)
        dchain(i)
        i = nc.vector.reciprocal(TN["stp"][g][:], TN["Lt"][g][:])
        dchain(i)
        nc.vector.tensor_scalar(TN["m2a"][g][:], TN["stp"][g][:], -2.0, None,
                                ALU.mult)
        dvi = nc.vector.tensor_scalar(TN["dv"][g][:], TN["stp"][g][:],
                                      2.0 * lam2, None, ALU.mult)
        E_dv = dchain(dvi)
        nc.vector.tensor_scalar(TN["ev"][g][:], TN["dv"][g][:], -1.0, 1.0,
                                ALU.mult, ALU.add)
        # theta = (1 - sqrt(q)) / (1 + sqrt(q)), q = 2*lam2*step
        wait("act", "dve", E_dv)
        sq = nc.scalar.activation(TN["sq"][g][:], TN["dv"][g][:], ACTF.Sqrt)
        E_sq = inc("act", sq)
        wait("dve", "act", E_sq)
        nc.vector.tensor_scalar(TN["onem"][g][:], TN["sq"][g][:], -1.0, 1.0,
                                ALU.mult, ALU.add)
        i = nc.vector.tensor_scalar(TN["onep"][g][:], TN["sq"][g][:], 1.0,
                                    None, ALU.add)
        dchain(i)
        i = nc.vector.reciprocal(TN["rop"][g][:], TN["onep"][g][:])
        dchain(i)
        nc.vector.tensor_tensor(TN["th"][g][:], TN["onem"][g][:],
                                TN["rop"][g][:], ALU.mult)
        # fv = step*(beta - lam1) + dv*wprev
        wait("dve", "dma_bw", E_bw)
        i = nc.vector.tensor_scalar(fv[g][:], beta_g[g][:], lam1,
                                    TN["stp"][g][:], ALU.subtract, ALU.mult)
        dchain(i)
        nc.vector.scalar_tensor_tensor(fv[g][:], wprev_g[g][:],
                                       TN["dv"][g][:], fv[g][:],
                                       ALU.mult, ALU.add)
        zi = nc.vector.tensor_copy(z[g][:], wA[g][:])
        E_zready[g] = inc("dve", zi)

    # ---------------- FISTA
    for ti in range(T):
        t = npow + 1 + ti
        wold = wA if ti % 2 == 0 else wB
        wnew = wB if ti % 2 == 0 else wA
        for g in range(ngrp):
            E_M = pe_transpose_and_matvec(t, g)
            dve_ysm(t, g, E_M)
            # v = ev*z - 2*step*y + fv ; vc = v - MAX_W
            i = nc.vector.scalar_tensor_tensor(v[g][:], ysm[g][:],
                                               TN["m2a"][g][:], fv[g][:],
                                               ALU.mult, ALU.add)
            dchain(i)
            i = nc.vector.scalar_tensor_tensor(v[g][:], z[g][:],
                                               TN["ev"][g][:], v[g][:],
                                               ALU.mult, ALU.add)
            dchain(i)
            i = nc.vector.tensor_scalar(vc[g][:], v[g][:], MAX_W, None,
                                        ALU.subtract)
            dchain(i)
            ni = NEWTON0 if ti == 0 else NEWTON
            for k in range(ni):
                # sum relu(v-tau) = sum max(v,tau) - P*tau, so
                # phi = sum[max(v,tau) - max(v-c,tau)] - 1 needs no ACT and
                # no negated-tau biases. Slope = c1+1 (damped; capped count
                # c2 is ~0 in practice). All DVE, 3 streaming ops.
                nc.vector.tensor_scalar(sc[g][:], v[g][:], TN["tau"][g][:],
                                        None, ALU.max, ALU.add,
                                        accum_out=TN["s1"][g][:])
                nc.vector.tensor_scalar(sd[g][:], vc[g][:], TN["tau"][g][:],
                                        None, ALU.max, ALU.add,
                                        accum_out=TN["s2"][g][:])
                c1i = nc.vector.tensor_scalar(sa[g][:], v[g][:],
                                              TN["tau"][g][:], None,
                                              ALU.is_gt, ALU.add,
                                              accum_out=TN["c1"][g][:])
                dchain(c1i)
                nc.vector.scalar_tensor_tensor(TN["phi"][g][:],
                                               TN["s1"][g][:], 1.0,
                                               TN["s2"][g][:], ALU.subtract,
                                               ALU.subtract)
                ci = nc.vector.tensor_scalar(TN["cnt"][g][:], TN["c1"][g][:],
                                             1.0, None, ALU.add)
                dchain(ci)
                i = nc.vector.reciprocal(TN["rc"][g][:], TN["cnt"][g][:])
                dchain(i)
                ta = nc.vector.scalar_tensor_tensor(TN["tau"][g][:],
                                                    TN["phi"][g][:],
                                                    TN["rc"][g][:],
                                                    TN["tau"][g][:],
                                                    ALU.mult, ALU.add)
                dchain(ta)
            # w_new = clip(v - tau, 0, MAX_W); dw = w_new - w_old
            i = nc.vector.tensor_scalar(t1[g][:], v[g][:], TN["tau"][g][:],
                                        0.0, ALU.subtract, ALU.max)
            dchain(i)
            wi = nc.vector.tensor_scalar(wnew[g][:], t1[g][:], MAX_W, None,
                                         ALU.min)
            dchain(wi)
            if ti < T - 1:
                i = nc.vector.scalar_tensor_tensor(dw[g][:], t1[g][:], MAX_W,
                                                   wold[g][:], ALU.min,
                                                   ALU.subtract)
                dchain(i)
                # z = w_new + th*dw
                zi = nc.vector.scalar_tensor_tensor(z[g][:], dw[g][:],
                                                    TN["th"][g][:],
                                                    wnew[g][:], ALU.mult,
                                                    ALU.add)
                E_zready[g] = inc("dve", zi)
            else:
                # renormalize and stage the output
                i = nc.vector.tensor_scalar(sd[g][:], wnew[g][:], 0.0, None,
                                            ALU.add, ALU.add,
                                            accum_out=TN["ssum"][g][:])
                dchain(i)
                i = nc.vector.tensor_scalar(TN["ssum"][g][:],
                                            TN["ssum"][g][:], EPS, None,
                                            ALU.add)
                dchain(i)
                i = nc.vector.reciprocal(TN["rs"][g][:], TN["ssum"][g][:])
                dchain(i)
                oi = nc.vector.tensor_scalar(outt[g][:], wnew[g][:],
                                             TN["rs"][g][:], None, ALU.mult)
                E_out_dve[g] = inc("dve", oi)

    # ---------------- store
    for g in range(ngrp):
        g0 = g * gb
        wait("sync", "dve", E_out_dve[g])
        d = nc.sync.dma_start(out=out_d[g0:g0 + gb, :], in_=outt[g][:])
        d.then_inc(sems["dma_out"], 16)
    nc.sync.wait_ge(sems["dma_out"], 16 * ngrp)


def build(lam1, lam2, nb=NB, npow=NPOW, T=T_FISTA):
    nc = bass.Bass("TRN2", target_bir_lowering=False, debug=False)
    sigma_d = nc.dram_tensor("sigma", [nb, P, P], F32, kind="ExternalInput")
    beta_d = nc.dram_tensor("beta", [nb, P], F32, kind="ExternalInput")
    wprev_d = nc.dram_tensor("w_prev", [nb, P], F32, kind="ExternalInput")
    out_d = nc.dram_tensor("out", [nb, P], F32, kind="ExternalOutput")
    with ExitStack() as ctx:
        _emit(ctx, nc, sigma_d.ap(), beta_d.ap(), wprev_d.ap(), out_d.ap(),
              lam1, lam2, nb, npow, T)
    return nc


def kernel(sigma, beta, w_prev, log_lambda1, log_lambda2):
    global LAST_RESULT
    sigma = np.ascontiguousarray(np.asarray(sigma, dtype=np.float32))
    beta = np.ascontiguousarray(np.asarray(beta, dtype=np.float32))
    w_prev = np.ascontiguousarray(np.asarray(w_prev, dtype=np.float32))
    lam1 = float(np.exp(np.float32(log_lambda1)))
    lam2 = float(np.exp(np.float32(log_lambda2)))

    nc = build(lam1, lam2)
    in_maps = []
    for c in range(N_CORES):
        s = slice(c * NB, (c + 1) * NB)
        in_maps.append({
            "sigma": sigma[s],
            "beta": beta[s],
            "w_prev": w_prev[s],
        })
    res = run_bass_kernel_spmd(nc, in_maps, list(range(N_CORES)), trace=TRACE)
    LAST_RESULT = res
    out = np.concatenate([res.results[c]["out"] for c in range(N_CORES)],
                         axis=0)
    return np.ascontiguousarray(out.astype(np.float32))



# revision 8
# speedup vs baseline: 5.6185x; 5.6185x over previous
"""Trainium2 Bass kernel for nn_DifferentiableRiskBudgeting.

Solves, per batch sample b:
    min_w  w' S_b w - beta_b' w + lam1*||w||_1 + lam2*||w - w_prev||^2
    s.t.   sum w = 1, 0 <= w <= MAX_W
then clamps + renormalizes — matching the reference's converged
projected-gradient solution. FISTA (T=20) with a warm-started 1-step
Newton projection per iteration replaces the reference's 250 PGD
iterations; validated in fp32 numpy against the reference to rel err
2.6e-3 (gate is 2e-2).

Sharding: pure data parallel, batch 512 = 64 samples per core on 8
cores, processed as two pipelined groups of 32 so the DVE projection
of one group overlaps the PE matvec path of the other.

Key structure (per group of 32 samples):
  - Power iterations (2) run entirely asset-major with NO
    normalization (power iteration is scale-free): PE matvec ->
    PSUM -> ACT copy to SBUF double buffer which IS the next round's
    transposed moving operand. Zero DVE work. A stale Rayleigh
    quotient (z_{k-1}.y_k / z_{k-1}.z_{k-1}) avoids an extra matvec.
  - FISTA rounds: DVE computes sample-major [32,256]; the per-sample
    step scale (-2*step) is folded into the PE z-transpose by using
    diag(-2*step) instead of the identity as the transpose matrix;
    the constant per-sample vector fv = step*(beta-lam1)+q*w_prev is
    added inside PSUM by one identity-stationary matmul per i-half,
    so v = ev*z + (S zs + fv) takes ONE DVE op.
  - y returns to sample-major via ACT copy (PSUM->SBUF) + PE
    transpose (SBUF->PSUM), keeping the 32x32 DVE StreamTransposes
    of the previous design off the critical engine.
  - Capped-simplex projection: 1 warm-started damped-Newton step on
    sum(clip(v-tau,0,c))=1 (slope = #(v>tau)+1); tau0 from the
    unconstrained solution (sum v - 1)/P on round 0 (2 steps there).

Raw bass (no Tile): this container's walrus build only allows ~2 sync
commands per instruction, which Tile's scheduler exceeds at every
cross-engine join. With explicit semaphores every wait is a standalone
single-wait instruction; all semaphore values are static because the
schedule is fully unrolled. Same-engine dependent ops also need a
producer-inc + consumer-wait pair (engine pipelines do not interlock),
with ordering transitive through any later same-engine inc.
"""

import math
import numpy as np
from contextlib import ExitStack

import concourse.bass as bass
from concourse import mybir
from concourse.bass_utils import run_bass_kernel_spmd

F32 = mybir.dt.float32
ALU = mybir.AluOpType
ACTF = mybir.ActivationFunctionType

B, P = 512, 256
N_CORES = 8
NB = B // N_CORES            # samples per core
HALF = P // 128              # sigma row-halves (2)
GB = 32                      # pipeline group size
NGRP = NB // GB
MAX_W = 0.1
EPS = 1e-8

NPOW = 2                     # scale-free power iterations
T_FISTA = 20                 # FISTA iterations
NEWTON0 = 2                  # Newton steps on the first projection
SAFETY = 1.4                 # L overestimation factor
SIG_DMA_BATCH = 4            # samples per sigma DMA

# set by the test harness; ignored by graders
TRACE = False
LAST_RESULT = None


def _emit(ctx, nc, sigma_d, beta_d, wprev_d, out_d, lam1, lam2):
    def sbuf(name, shape):
        return ctx.enter_context(nc.sbuf_tensor(name, shape, F32))

    def psum(name):
        # full-bank tensors so PE writes and DVE reads of different
        # buffers can never share a PSUM bank (fatal on HW)
        return ctx.enter_context(nc.psum_tensor(name, [128, 512], F32))

    sem_names = ["pe", "act", "dve", "pool", "dma_bw", "dma_out"]
    nk = (NB + SIG_DMA_BATCH - 1) // SIG_DMA_BATCH
    sem_names += [f"dsig{k}" for k in range(nk)]
    sems = {e: ctx.enter_context(nc.semaphore(f"s_{e}")) for e in sem_names}
    ENG = {"pe": nc.tensor, "dve": nc.vector, "act": nc.scalar,
           "pool": nc.gpsimd, "sync": nc.sync}
    ctr = {e: 0 for e in sems}
    last_wait = {e: {} for e in list(ENG)}

    def inc(ename, inst, n=1):
        ctr[ename] += n
        inst.then_inc(sems[ename], n)
        return ctr[ename]

    def wait(consumer, producer, value):
        if value is None or value <= 0:
            return
        lw = last_wait[consumer]
        if lw.get(producer, 0) >= value:
            return
        ENG[consumer].wait_ge(sems[producer], value)
        lw[producer] = value

    def dchain(inst):
        t = inc("dve", inst)
        wait("dve", "dve", t)
        return t

    # ---------------- tensors
    ident = sbuf("ident", [128, 128])
    nbatch = SIG_DMA_BATCH
    sig = [sbuf(f"sig{k}", [128, nbatch * HALF * P]) for k in range(nk)]

    def sig_ap(b, hj, hi):
        k, m = divmod(b, nbatch)
        c0 = (m * HALF + hj) * P + hi * 128
        return sig[k][:, c0:c0 + 128]

    def gt(name, shape):
        return [sbuf(f"{name}{g}", shape) for g in range(NGRP)]

    z = gt("z", [GB, P])
    v = gt("v", [GB, P])
    t1 = gt("t1", [GB, P])
    wA = gt("wA", [GB, P])
    wB = gt("wB", [GB, P])
    dw = gt("dw", [GB, P])
    fv = gt("fv", [GB, P])
    beta_g = gt("beta", [GB, P])
    wprev_g = gt("wprev", [GB, P])
    outt = gt("outt", [GB, P])
    dum = gt("dum", [GB, P])         # elementwise discard for accum ops
    zsm_sb = gt("zsm", [GB, P])      # sample-major z for the Rayleigh
    zTc = sbuf("zTc", [128, GB])     # all-ones moving operand, power r=0
    ybuf = [[sbuf(f"ybuf{g}_{p}", [128, HALF * GB]) for p in range(2)]
            for g in range(NGRP)]
    zT = [[sbuf(f"zT{g}_{h}", [128, GB]) for h in range(HALF)]
          for g in range(NGRP)]
    fvT = [[sbuf(f"fvT{g}_{h}", [128, GB]) for h in range(HALF)]
           for g in range(NGRP)]
    dm = gt("dm", [GB, GB])          # diag(-2*step) transpose matrix
    tiny_names = ("tau tauc s1 s2 c1 phi cnt rc num den rden lmax Lt stp "
                  "m2a dv ev sq onem onep rop th sv ssum rs")
    TN = {}
    for name in tiny_names.split():
        TN[name] = gt(name, [GB, 1])

    ptb = [psum(f"pt{g}") for g in range(NGRP)]    # z/fv transposes + zsm
    yb = [psum(f"y{g}") for g in range(NGRP)]      # matvec accumulator
    ysb = [psum(f"ys{g}") for g in range(NGRP)]    # sample-major y

    # ---------------- events (per group)
    E_z = [0] * NGRP            # dve: z ready for transpose
    E_ptfree = [[] for _ in range(NGRP)]   # [(engine, tick)]: pt consumed
    E_ycopy = [0] * NGRP        # act: yp copied out (ybank free)
    E_ysmfree = [0] * NGRP      # dve: ysm consumed
    E_ybufread = [[0, 0] for _ in range(NGRP)]  # pe: ybuf[parity] read
    E_out = [0] * NGRP

    # ---------------- preamble
    mz = nc.vector.memset(ident[:], 0.0)
    E_identz = inc("dve", mz)
    wait("pool", "dve", E_identz)
    af = nc.gpsimd.affine_select(
        out=ident[:], in_=ident[:], compare_op=ALU.not_equal, fill=1.0,
        base=0, pattern=[[-1, 128]], channel_multiplier=1)
    E_ident = inc("pool", af)

    for k in range(nk):
        kn = min(nbatch, NB - k * nbatch)
        srca = sigma_d[k * nbatch:k * nbatch + kn].rearrange(
            "b (h p) j -> p b h j", p=128)
        dst = sig[k][:].rearrange("p (b h j) -> p b h j", b=kn, h=HALF)
        d = nc.sync.dma_start(out=dst, in_=srca)
        d.then_inc(sems[f"dsig{k}"], 16)
    for g in range(NGRP):
        g0 = g * GB
        d = nc.sync.dma_start(out=beta_g[g][:], in_=beta_d[g0:g0 + GB, :])
        d.then_inc(sems["dma_bw"], 16)
        d = nc.sync.dma_start(out=wprev_g[g][:], in_=wprev_d[g0:g0 + GB, :])
        d.then_inc(sems["dma_bw"], 16)
    E_bw = 32 * NGRP

    nc.vector.memset(zTc[:], 1.0)
    for g in range(NGRP):
        nc.vector.memset(z[g][:], 1.0 / P)
        m = nc.vector.memset(wA[g][:], 1.0 / P)
        E_z[g] = inc("dve", m)
    E_zTc = E_z[NGRP - 1]

    # ---------------- helpers
    def matvec(g, moving, with_fv, dma_gate, yfree_tick):
        """PE: 128 (+2) matmuls accumulating y[i, hi*GB+s] into yb[g]."""
        wait("pe", "act", yfree_tick)
        if with_fv:
            # start=True clears the whole bank's has_written bits, so only
            # the first block may carry it; the second overwrites (hw=0).
            for hi in range(HALF):
                nc.tensor.matmul(yb[g][:, hi * GB:(hi + 1) * GB],
                                 ident[:, :], fvT[g][hi][:, :],
                                 start=(hi == 0), stop=False)
        g0 = g * GB
        mm = None
        for bb in range(GB):
            b = g0 + bb
            if dma_gate:
                wait("pe", f"dsig{b // nbatch}", 16)
            for hi in range(HALF):
                for hj in range(HALF):
                    mm = nc.tensor.matmul(
                        yb[g][:, hi * GB + bb:hi * GB + bb + 1],
                        sig_ap(b, hj, hi),
                        moving(hj, bb),
                        start=(hj == 0 and not with_fv),
                        stop=(hj == HALF - 1))
        return inc("pe", mm)

    # ---------------- power iterations (asset-major, no normalization)
    E_mm = [0] * NGRP
    for r in range(NPOW):
        for g in range(NGRP):
            if r == 0:
                wait("pe", "dve", E_zTc)
                mov = lambda hj, bb: zTc[:, bb:bb + 1]
            else:
                prev = ybuf[g][(r - 1) % 2]
                mov = (lambda pv: lambda hj, bb:
                       pv[:, hj * GB + bb:hj * GB + bb + 1])(prev)
                wait("pe", "act", E_ycopy[g])
            E_mm[g] = matvec(g, mov, with_fv=False, dma_gate=(r == 0),
                             yfree_tick=E_ycopy[g])
        for g in range(NGRP):
            wait("act", "pe", E_mm[g])
            cp = nc.scalar.copy(ybuf[g][r % 2][:, :], yb[g][:, 0:HALF * GB])
            E_ycopy[g] = inc("act", cp)

    # ---------------- Rayleigh quotient + FISTA coefficients
    E_ray = [0] * NGRP
    for g in range(NGRP):
        wait("pe", "act", E_ycopy[g])
        wait("pe", "pool", E_ident)
        tr = None
        for hi in range(HALF):
            # zsm: matvec input of the last power round
            tr = nc.tensor.transpose(
                ptb[g][0:GB, hi * 128:(hi + 1) * 128],
                ybuf[g][(NPOW - 2) % 2][:, hi * GB:(hi + 1) * GB],
                ident[:, :])
        for hi in range(HALF):
            tr = nc.tensor.transpose(
                ysb[g][0:GB, hi * 128:(hi + 1) * 128],
                ybuf[g][(NPOW - 1) % 2][:, hi * GB:(hi + 1) * GB],
                ident[:, :])
        E_ray[g] = inc("pe", tr)
        E_ybufread[g][0] = E_ybufread[g][1] = E_ray[g]

    E_zsm = [0] * NGRP
    for g in range(NGRP):
        # DVE cannot read two PSUM banks in one op: stage zsm via ACT
        wait("act", "pe", E_ray[g])
        cp = nc.scalar.copy(zsm_sb[g][:], ptb[g][0:GB, 0:P])
        E_zsm[g] = inc("act", cp)

    for g in range(NGRP):
        ysm = ysb[g][0:GB, 0:P]
        wait("dve", "act", E_zsm[g])
        wait("dve", "pe", E_ray[g])
        nc.vector.scalar_tensor_tensor(dum[g][:], zsm_sb[g][:], 1.0, ysm,
                                       ALU.mult, ALU.mult,
                                       accum_out=TN["num"][g][:])
        i = nc.vector.scalar_tensor_tensor(dum[g][:], zsm_sb[g][:], 1.0,
                                           zsm_sb[g][:],
                                           ALU.mult, ALU.mult,
                                           accum_out=TN["den"][g][:])
        E_ysmfree[g] = dchain(i)

    for g in range(NGRP):
        i = nc.vector.tensor_scalar(TN["den"][g][:], TN["den"][g][:], EPS,
                                    None, ALU.add)
        dchain(i)
        i = nc.vector.reciprocal(TN["rden"][g][:], TN["den"][g][:])
        dchain(i)
        i = nc.vector.tensor_tensor(TN["lmax"][g][:], TN["num"][g][:],
                                    TN["rden"][g][:], ALU.mult)
        dchain(i)
        i = nc.vector.tensor_scalar(TN["Lt"][g][:], TN["lmax"][g][:],
                                    2.0 * SAFETY, SAFETY * 2.0 * lam2,
                                    ALU.mult, ALU.add)
        dchain(i)
        i = nc.vector.reciprocal(TN["stp"][g][:], TN["Lt"][g][:])
        dchain(i)
        nc.vector.tensor_scalar(TN["m2a"][g][:], TN["stp"][g][:], -2.0, None,
                                ALU.mult)
        dvi = nc.vector.tensor_scalar(TN["dv"][g][:], TN["stp"][g][:],
                                      2.0 * lam2, None, ALU.mult)
        E_dv = dchain(dvi)
        nc.vector.tensor_scalar(TN["ev"][g][:], TN["dv"][g][:], -1.0, 1.0,
                                ALU.mult, ALU.add)
        # theta = (1 - sqrt(q)) / (1 + sqrt(q)), q = 2*lam2*step
        wait("act", "dve", E_dv)
        sq = nc.scalar.activation(TN["sq"][g][:], TN["dv"][g][:], ACTF.Sqrt)
        E_sq = inc("act", sq)
        wait("dve", "act", E_sq)
        nc.vector.tensor_scalar(TN["onem"][g][:], TN["sq"][g][:], -1.0, 1.0,
                                ALU.mult, ALU.add)
        i = nc.vector.tensor_scalar(TN["onep"][g][:], TN["sq"][g][:], 1.0,
                                    None, ALU.add)
        dchain(i)
        i = nc.vector.reciprocal(TN["rop"][g][:], TN["onep"][g][:])
        dchain(i)
        nc.vector.tensor_tensor(TN["th"][g][:], TN["onem"][g][:],
                                TN["rop"][g][:], ALU.mult)
        # fv = step*(beta - lam1) + q*w_prev ; dm = diag(-2*step)
        wait("dve", "dma_bw", E_bw)
        i = nc.vector.tensor_scalar(fv[g][:], beta_g[g][:], lam1,
                                    TN["stp"][g][:], ALU.subtract, ALU.mult)
        dchain(i)
        nc.vector.scalar_tensor_tensor(fv[g][:], wprev_g[g][:],
                                       TN["dv"][g][:], fv[g][:],
                                       ALU.mult, ALU.add)
        i = nc.vector.tensor_scalar(dm[g][:], ident[0:GB, 0:GB],
                                    TN["m2a"][g][:], None, ALU.mult)
        E_fv = dchain(i)
        # fvT via PE transpose (identity) + ACT copy back to SBUF
        wait("pe", "dve", E_fv)
        tr = None
        for h in range(HALF):
            tr = nc.tensor.transpose(
                ptb[g][:, 2 * GB + h * GB:2 * GB + (h + 1) * GB],
                fv[g][:, h * 128:(h + 1) * 128],
                ident[0:GB, 0:GB])
        E_fvT = inc("pe", tr)
        wait("act", "pe", E_fvT)
        cp = None
        for h in range(HALF):
            cp = nc.scalar.copy(fvT[g][h][:, :],
                                ptb[g][:, 2 * GB + h * GB:2 * GB + (h + 1) * GB])
        E_fvTc = inc("act", cp)
        E_ptfree[g] = [("act", E_fvTc), ("dve", E_fv)]

    # ---------------- FISTA
    for ti in range(T_FISTA):
        wold = wA if ti % 2 == 0 else wB
        wnew = wB if ti % 2 == 0 else wA
        E_pt = [0] * NGRP
        E_zT = [0] * NGRP
        E_ysm = [0] * NGRP
        # PE: z -> zT (scaled by diag(-2*step)) ; ACT: psum -> SBUF
        for g in range(NGRP):
            wait("pe", "dve", E_z[g])
            for eng, tick in E_ptfree[g]:
                wait("pe", eng, tick)
            tr = None
            for h in range(HALF):
                # regular matmul (NOT transpose: the PE transpose datapath
                # ignores the matrix values): zT = z^T @ diag(-2*step)
                tr = nc.tensor.matmul(
                    ptb[g][:, h * GB:(h + 1) * GB],
                    z[g][:, h * 128:(h + 1) * 128],
                    dm[g][:, :], start=True, stop=True)
            E_pt[g] = inc("pe", tr)
        for g in range(NGRP):
            wait("act", "pe", E_pt[g])
            cp = None
            for h in range(HALF):
                cp = nc.scalar.copy(zT[g][h][:, :],
                                    ptb[g][:, h * GB:(h + 1) * GB])
            E_zT[g] = inc("act", cp)
            E_ptfree[g] = [("act", E_zT[g])]
        # PE: matvec with fv folded in
        for g in range(NGRP):
            wait("pe", "act", E_zT[g])
            mov = (lambda gg: lambda hj, bb: zT[gg][hj][:, bb:bb + 1])(g)
            E_mm[g] = matvec(g, mov, with_fv=True, dma_gate=False,
                             yfree_tick=E_ycopy[g])
        # ACT: yp -> SBUF staging ; PE: staging -> sample-major ysm
        for g in range(NGRP):
            wait("act", "pe", E_mm[g])
            stage = ybuf[g][ti % 2]
            wait("act", "pe", E_ybufread[g][ti % 2])
            cp = nc.scalar.copy(stage[:, :], yb[g][:, 0:HALF * GB])
            E_ycopy[g] = inc("act", cp)
        for g in range(NGRP):
            stage = ybuf[g][ti % 2]
            wait("pe", "act", E_ycopy[g])
            wait("pe", "dve", E_ysmfree[g])
            tr = None
            for hi in range(HALF):
                tr = nc.tensor.transpose(
                    ysb[g][0:GB, hi * 128:(hi + 1) * 128],
                    stage[:, hi * GB:(hi + 1) * GB],
                    ident[:, :])
            E_ysm[g] = inc("pe", tr)
            E_ybufread[g][ti % 2] = E_ysm[g]
        # DVE: v = ev*z + (S*(-2*step*z) + fv)
        for g in range(NGRP):
            ysm = ysb[g][0:GB, 0:P]
            wait("dve", "pe", E_ysm[g])
            i = nc.vector.scalar_tensor_tensor(
                v[g][:], z[g][:], TN["ev"][g][:], ysm, ALU.mult, ALU.add,
                accum_out=(TN["sv"][g][:] if ti == 0 else None))
            E_ysmfree[g] = dchain(i)
        if ti == 0:
            for g in range(NGRP):
                i = nc.vector.tensor_scalar(TN["tau"][g][:], TN["sv"][g][:],
                                            1.0, 1.0 / P, ALU.subtract,
                                            ALU.mult)
                dchain(i)
                i = nc.vector.tensor_scalar(TN["tauc"][g][:], TN["tau"][g][:],
                                            MAX_W, None, ALU.add)
                dchain(i)
        # Newton step(s) on sum(clip(v - tau, 0, c)) = 1
        for it_n in range(NEWTON0 if ti == 0 else 1):
            for g in range(NGRP):
                nc.vector.tensor_scalar(dum[g][:], v[g][:], TN["tau"][g][:],
                                        None, ALU.max, ALU.add,
                                        accum_out=TN["s1"][g][:])
                nc.vector.tensor_scalar(dum[g][:], v[g][:], TN["tauc"][g][:],
                                        None, ALU.max, ALU.add,
                                        accum_out=TN["s2"][g][:])
                i = nc.vector.tensor_scalar(dum[g][:], v[g][:],
                                            TN["tau"][g][:], None,
                                            ALU.is_gt, ALU.add,
                                            accum_out=TN["c1"][g][:])
                dchain(i)
            for g in range(NGRP):
                nc.vector.scalar_tensor_tensor(
                    TN["phi"][g][:], TN["s1"][g][:], -(P * MAX_W - 1.0),
                    TN["s2"][g][:], ALU.subtract, ALU.subtract)
                i = nc.vector.tensor_scalar(TN["cnt"][g][:], TN["c1"][g][:],
                                            1.0, None, ALU.add)
                dchain(i)
                i = nc.vector.reciprocal(TN["rc"][g][:], TN["cnt"][g][:])
                dchain(i)
                i = nc.vector.scalar_tensor_tensor(
                    TN["tau"][g][:], TN["phi"][g][:], TN["rc"][g][:],
                    TN["tau"][g][:], ALU.mult, ALU.add)
                dchain(i)
                i = nc.vector.tensor_scalar(TN["tauc"][g][:], TN["tau"][g][:],
                                            MAX_W, None, ALU.add)
                dchain(i)
        # w_new = clip(v - tau, 0, MAX_W)
        for g in range(NGRP):
            i = nc.vector.tensor_scalar(t1[g][:], v[g][:], TN["tau"][g][:],
                                        0.0, ALU.subtract, ALU.max)
            dchain(i)
        if ti < T_FISTA - 1:
            for g in range(NGRP):
                nc.vector.tensor_scalar(wnew[g][:], t1[g][:], MAX_W, None,
                                        ALU.min)
                i = nc.vector.scalar_tensor_tensor(
                    dw[g][:], t1[g][:], MAX_W, wold[g][:],
                    ALU.min, ALU.subtract)
                dchain(i)
                zi = nc.vector.scalar_tensor_tensor(
                    z[g][:], dw[g][:], TN["th"][g][:], wnew[g][:],
                    ALU.mult, ALU.add)
                E_z[g] = inc("dve", zi)
        else:
            # renormalize and stage the output
            for g in range(NGRP):
                i = nc.vector.tensor_scalar(wnew[g][:], t1[g][:], MAX_W, None,
                                            ALU.min, ALU.add,
                                            accum_out=TN["ssum"][g][:])
                dchain(i)
                i = nc.vector.tensor_scalar(TN["ssum"][g][:],
                                            TN["ssum"][g][:], EPS, None,
                                            ALU.add)
                dchain(i)
                i = nc.vector.reciprocal(TN["rs"][g][:], TN["ssum"][g][:])
                dchain(i)
                oi = nc.vector.tensor_scalar(outt[g][:], wnew[g][:],
                                             TN["rs"][g][:], None, ALU.mult)
                E_out[g] = inc("dve", oi)

    # ---------------- store
    for g in range(NGRP):
        g0 = g * GB
        wait("sync", "dve", E_out[g])
        d = nc.sync.dma_start(out=out_d[g0:g0 + GB, :], in_=outt[g][:])
        d.then_inc(sems["dma_out"], 16)
    nc.sync.wait_ge(sems["dma_out"], 16 * NGRP)


def build(lam1, lam2):
    nc = bass.Bass("TRN2", target_bir_lowering=False, debug=False)
    sigma_d = nc.dram_tensor("sigma", [NB, P, P], F32, kind="ExternalInput")
    beta_d = nc.dram_tensor("beta", [NB, P], F32, kind="ExternalInput")
    wprev_d = nc.dram_tensor("w_prev", [NB, P], F32, kind="ExternalInput")
    out_d = nc.dram_tensor("out", [NB, P], F32, kind="ExternalOutput")
    with ExitStack() as ctx:
        _emit(ctx, nc, sigma_d.ap(), beta_d.ap(), wprev_d.ap(), out_d.ap(),
              lam1, lam2)
    return nc


def kernel(sigma, beta, w_prev, log_lambda1, log_lambda2):
    global LAST_RESULT
    sigma = np.ascontiguousarray(np.asarray(sigma, dtype=np.float32))
    beta = np.ascontiguousarray(np.asarray(beta, dtype=np.float32))
    w_prev = np.ascontiguousarray(np.asarray(w_prev, dtype=np.float32))
    lam1 = float(np.exp(np.float32(log_lambda1)))
    lam2 = float(np.exp(np.float32(log_lambda2)))

    nc = build(lam1, lam2)
    in_maps = []
    for c in range(N_CORES):
        s = slice(c * NB, (c + 1) * NB)
        in_maps.append({
            "sigma": sigma[s],
            "beta": beta[s],
            "w_prev": w_prev[s],
        })
    res = run_bass_kernel_spmd(nc, in_maps, list(range(N_CORES)), trace=TRACE)
    LAST_RESULT = res
    out = np.concatenate([res.results[c]["out"] for c in range(N_CORES)],
                         axis=0)
    return np.ascontiguousarray(out.astype(np.float32))


# revision 10
# speedup vs baseline: 7.1967x; 1.2809x over previous
"""Trainium2 Bass kernel for nn_DifferentiableRiskBudgeting.

Solves, per batch sample b:
    min_w  w' S_b w - beta_b' w + lam1*||w||_1 + lam2*||w - w_prev||^2
    s.t.   sum w = 1, 0 <= w <= MAX_W
then clamps + renormalizes — matching the reference's converged
projected-gradient solution. FISTA (T=20) with a warm-started 1-step
Newton projection per iteration replaces the reference's 250 PGD
iterations; validated in fp32 numpy against the reference to rel err
2.6e-3 (gate is 2e-2).

Sharding: pure data parallel, batch 512 = 64 samples per core on 8
cores, processed as two pipelined groups of 32 so the DVE projection
of one group overlaps the PE matvec path of the other.

Key structure (per group of 32 samples):
  - Power iterations (2) run entirely asset-major with NO
    normalization (power iteration is scale-free): PE matvec ->
    PSUM -> ACT copy to SBUF double buffer which IS the next round's
    transposed moving operand. Zero DVE work. A stale Rayleigh
    quotient (z_{k-1}.y_k / z_{k-1}.z_{k-1}) avoids an extra matvec.
  - FISTA rounds: DVE computes sample-major [32,256]; the per-sample
    step scale (-2*step) is folded into the PE z-transpose by using
    diag(-2*step) instead of the identity as the transpose matrix;
    the constant per-sample vector fv = step*(beta-lam1)+q*w_prev is
    added inside PSUM by one identity-stationary matmul per i-half,
    so v = ev*z + (S zs + fv) takes ONE DVE op.
  - y returns to sample-major via ACT copy (PSUM->SBUF) + PE
    transpose (SBUF->PSUM), keeping the 32x32 DVE StreamTransposes
    of the previous design off the critical engine.
  - Capped-simplex projection: 1 warm-started damped-Newton step on
    sum(clip(v-tau,0,c))=1 (slope = #(v>tau)+1); tau0 from the
    unconstrained solution (sum v - 1)/P on round 0 (2 steps there).

Raw bass (no Tile): this container's walrus build only allows ~2 sync
commands per instruction, which Tile's scheduler exceeds at every
cross-engine join. With explicit semaphores every wait is a standalone
single-wait instruction; all semaphore values are static because the
schedule is fully unrolled. Same-engine dependent ops also need a
producer-inc + consumer-wait pair (engine pipelines do not interlock),
with ordering transitive through any later same-engine inc.
"""

import math
import numpy as np
from contextlib import ExitStack

import concourse.bass as bass
from concourse import mybir
from concourse.bass_utils import run_bass_kernel_spmd

F32 = mybir.dt.float32
F16 = mybir.dt.float16
ALU = mybir.AluOpType
ACTF = mybir.ActivationFunctionType

B, P = 512, 256
N_CORES = 8
NB = B // N_CORES            # samples per core
HALF = P // 128              # sigma row-halves (2)
GB = 32                      # pipeline group size
NGRP = NB // GB
MAX_W = 0.1
EPS = 1e-8

NPOW = 2                     # scale-free power iterations
T_FISTA = 20                 # FISTA iterations
NEWTON0 = 2                  # Newton steps on the first projection
SAFETY = 1.4                 # L overestimation factor
SIG_DMA_BATCH = 4            # samples per sigma DMA

# set by the test harness; ignored by graders
TRACE = False
LAST_RESULT = None


def _emit(ctx, nc, sigma_d, beta_d, wprev_d, out_d, lam1, lam2):
    def sbuf(name, shape):
        return ctx.enter_context(nc.sbuf_tensor(name, shape, F32))

    def psum(name):
        # full-bank tensors so PE writes and DVE reads of different
        # buffers can never share a PSUM bank (fatal on HW)
        return ctx.enter_context(nc.psum_tensor(name, [128, 512], F32))

    sem_names = ["pe", "act", "dve", "pool", "dma_bw", "dma_out"]
    nk = (NB + SIG_DMA_BATCH - 1) // SIG_DMA_BATCH
    sem_names += [f"dsig{k}" for k in range(nk)]
    sems = {e: ctx.enter_context(nc.semaphore(f"s_{e}")) for e in sem_names}
    ENG = {"pe": nc.tensor, "dve": nc.vector, "act": nc.scalar,
           "pool": nc.gpsimd, "sync": nc.sync}
    ctr = {e: 0 for e in sems}
    last_wait = {e: {} for e in list(ENG)}

    def inc(ename, inst, n=1):
        ctr[ename] += n
        inst.then_inc(sems[ename], n)
        return ctr[ename]

    def wait(consumer, producer, value):
        if value is None or value <= 0:
            return
        lw = last_wait[consumer]
        if lw.get(producer, 0) >= value:
            return
        ENG[consumer].wait_ge(sems[producer], value)
        lw[producer] = value

    def dchain(inst):
        t = inc("dve", inst)
        wait("dve", "dve", t)
        return t

    # ---------------- tensors
    ident = sbuf("ident", [128, 128])
    nbatch = SIG_DMA_BATCH
    sig = [sbuf(f"sig{k}", [128, nbatch * HALF * P]) for k in range(nk)]

    def sig_ap(b, hj, hi):
        k, m = divmod(b, nbatch)
        c0 = (m * HALF + hj) * P + hi * 128
        return sig[k][:, c0:c0 + 128]

    def gt(name, shape):
        return [sbuf(f"{name}{g}", shape) for g in range(NGRP)]

    def gt16(name, shape):
        return [ctx.enter_context(nc.sbuf_tensor(f"{name}{g}", shape, F16))
                for g in range(NGRP)]

    z = gt16("z", [GB, P])
    v = gt16("v", [GB, P])
    t1 = gt16("t1", [GB, P])
    wA = gt16("wA", [GB, P])
    wB = gt16("wB", [GB, P])
    dw = gt16("dw", [GB, P])
    fv = gt("fv", [GB, P])
    beta_g = gt("beta", [GB, P])
    wprev_g = gt("wprev", [GB, P])
    outt = gt("outt", [GB, P])
    dum = gt16("dum", [GB, P])       # elementwise discard for accum ops
    zsm_sb = gt("zsm", [GB, P])      # sample-major z for the Rayleigh
    zTc = sbuf("zTc", [128, GB])     # all-ones moving operand, power r=0
    ybuf = [[sbuf(f"ybuf{g}_{p}", [128, HALF * GB]) for p in range(2)]
            for g in range(NGRP)]
    zT = [[sbuf(f"zT{g}_{h}", [128, GB]) for h in range(HALF)]
          for g in range(NGRP)]
    fvT = [[sbuf(f"fvT{g}_{h}", [128, GB]) for h in range(HALF)]
           for g in range(NGRP)]
    dm = [ctx.enter_context(nc.sbuf_tensor(f"dm{g}", [GB, GB], F16))
          for g in range(NGRP)]     # diag(-2*step) transpose matrix
    tiny_names = ("tau tauc s1 s2 c1 phi cnt rc num den rden lmax Lt stp "
                  "m2a dv ev sq onem onep rop th sv ssum rs")
    TN = {}
    for name in tiny_names.split():
        TN[name] = gt(name, [GB, 1])

    ptb = [psum(f"pt{g}") for g in range(NGRP)]    # z/fv transposes + zsm
    yb = [psum(f"y{g}") for g in range(NGRP)]      # matvec accumulator
    ysb = [psum(f"ys{g}") for g in range(NGRP)]    # sample-major y

    # ---------------- events (per group)
    E_z = [0] * NGRP            # dve: z ready for transpose
    E_ptfree = [[] for _ in range(NGRP)]   # [(engine, tick)]: pt consumed
    E_ycopy = [0] * NGRP        # act: yp copied out (ybank free)
    E_ysmfree = [0] * NGRP      # dve: ysm consumed
    E_ybufread = [[0, 0] for _ in range(NGRP)]  # pe: ybuf[parity] read
    E_out = [0] * NGRP

    # ---------------- preamble
    mz = nc.vector.memset(ident[:], 0.0)
    E_identz = inc("dve", mz)
    wait("pool", "dve", E_identz)
    af = nc.gpsimd.affine_select(
        out=ident[:], in_=ident[:], compare_op=ALU.not_equal, fill=1.0,
        base=0, pattern=[[-1, 128]], channel_multiplier=1)
    E_ident = inc("pool", af)

    for k in range(nk):
        kn = min(nbatch, NB - k * nbatch)
        srca = sigma_d[k * nbatch:k * nbatch + kn].rearrange(
            "b (h p) j -> p b h j", p=128)
        dst = sig[k][:].rearrange("p (b h j) -> p b h j", b=kn, h=HALF)
        d = nc.sync.dma_start(out=dst, in_=srca)
        d.then_inc(sems[f"dsig{k}"], 16)
    for g in range(NGRP):
        g0 = g * GB
        d = nc.sync.dma_start(out=beta_g[g][:], in_=beta_d[g0:g0 + GB, :])
        d.then_inc(sems["dma_bw"], 16)
        d = nc.sync.dma_start(out=wprev_g[g][:], in_=wprev_d[g0:g0 + GB, :])
        d.then_inc(sems["dma_bw"], 16)
    E_bw = 32 * NGRP

    nc.vector.memset(zTc[:], 1.0)
    for g in range(NGRP):
        nc.vector.memset(z[g][:], 1.0 / P)
        m = nc.vector.memset(wA[g][:], 1.0 / P)
        E_z[g] = inc("dve", m)
    E_zTc = E_z[NGRP - 1]

    # ---------------- helpers
    def matvec(g, moving, with_fv, dma_gate, yfree_tick):
        """PE: 128 (+2) matmuls accumulating y[i, hi*GB+s] into yb[g]."""
        wait("pe", "act", yfree_tick)
        if with_fv:
            # start=True clears the whole bank's has_written bits, so only
            # the first block may carry it; the second overwrites (hw=0).
            for hi in range(HALF):
                nc.tensor.matmul(yb[g][:, hi * GB:(hi + 1) * GB],
                                 ident[:, :], fvT[g][hi][:, :],
                                 start=(hi == 0), stop=False)
        g0 = g * GB
        mm = None
        for bb in range(GB):
            b = g0 + bb
            if dma_gate:
                wait("pe", f"dsig{b // nbatch}", 16)
            for hi in range(HALF):
                for hj in range(HALF):
                    mm = nc.tensor.matmul(
                        yb[g][:, hi * GB + bb:hi * GB + bb + 1],
                        sig_ap(b, hj, hi),
                        moving(hj, bb),
                        start=(hj == 0 and not with_fv),
                        stop=(hj == HALF - 1))
        return inc("pe", mm)

    # ---------------- power iterations (asset-major, no normalization)
    E_mm = [0] * NGRP
    for r in range(NPOW):
        for g in range(NGRP):
            if r == 0:
                wait("pe", "dve", E_zTc)
                mov = lambda hj, bb: zTc[:, bb:bb + 1]
            else:
                prev = ybuf[g][(r - 1) % 2]
                mov = (lambda pv: lambda hj, bb:
                       pv[:, hj * GB + bb:hj * GB + bb + 1])(prev)
                wait("pe", "act", E_ycopy[g])
            E_mm[g] = matvec(g, mov, with_fv=False, dma_gate=(r == 0),
                             yfree_tick=E_ycopy[g])
        for g in range(NGRP):
            wait("act", "pe", E_mm[g])
            cp = nc.scalar.copy(ybuf[g][r % 2][:, :], yb[g][:, 0:HALF * GB])
            E_ycopy[g] = inc("act", cp)

    # ---------------- Rayleigh quotient + FISTA coefficients
    E_ray = [0] * NGRP
    for g in range(NGRP):
        wait("pe", "act", E_ycopy[g])
        wait("pe", "pool", E_ident)
        tr = None
        for hi in range(HALF):
            # zsm: matvec input of the last power round
            tr = nc.tensor.transpose(
                ptb[g][0:GB, hi * 128:(hi + 1) * 128],
                ybuf[g][(NPOW - 2) % 2][:, hi * GB:(hi + 1) * GB],
                ident[:, :])
        for hi in range(HALF):
            tr = nc.tensor.transpose(
                ysb[g][0:GB, hi * 128:(hi + 1) * 128],
                ybuf[g][(NPOW - 1) % 2][:, hi * GB:(hi + 1) * GB],
                ident[:, :])
        E_ray[g] = inc("pe", tr)
        E_ybufread[g][0] = E_ybufread[g][1] = E_ray[g]

    E_zsm = [0] * NGRP
    for g in range(NGRP):
        # DVE cannot read two PSUM banks in one op: stage zsm via ACT
        wait("act", "pe", E_ray[g])
        cp = nc.scalar.copy(zsm_sb[g][:], ptb[g][0:GB, 0:P])
        E_zsm[g] = inc("act", cp)

    for g in range(NGRP):
        ysm = ysb[g][0:GB, 0:P]
        wait("dve", "act", E_zsm[g])
        wait("dve", "pe", E_ray[g])
        nc.vector.scalar_tensor_tensor(dum[g][:], zsm_sb[g][:], 1.0, ysm,
                                       ALU.mult, ALU.mult,
                                       accum_out=TN["num"][g][:])
        i = nc.vector.scalar_tensor_tensor(dum[g][:], zsm_sb[g][:], 1.0,
                                           zsm_sb[g][:],
                                           ALU.mult, ALU.mult,
                                           accum_out=TN["den"][g][:])
        E_ysmfree[g] = dchain(i)

    for g in range(NGRP):
        i = nc.vector.tensor_scalar(TN["den"][g][:], TN["den"][g][:], EPS,
                                    None, ALU.add)
        dchain(i)
        i = nc.vector.reciprocal(TN["rden"][g][:], TN["den"][g][:])
        dchain(i)
        i = nc.vector.tensor_tensor(TN["lmax"][g][:], TN["num"][g][:],
                                    TN["rden"][g][:], ALU.mult)
        dchain(i)
        i = nc.vector.tensor_scalar(TN["Lt"][g][:], TN["lmax"][g][:],
                                    2.0 * SAFETY, SAFETY * 2.0 * lam2,
                                    ALU.mult, ALU.add)
        dchain(i)
        i = nc.vector.reciprocal(TN["stp"][g][:], TN["Lt"][g][:])
        dchain(i)
        nc.vector.tensor_scalar(TN["m2a"][g][:], TN["stp"][g][:], -2.0, None,
                                ALU.mult)
        dvi = nc.vector.tensor_scalar(TN["dv"][g][:], TN["stp"][g][:],
                                      2.0 * lam2, None, ALU.mult)
        E_dv = dchain(dvi)
        nc.vector.tensor_scalar(TN["ev"][g][:], TN["dv"][g][:], -1.0, 1.0,
                                ALU.mult, ALU.add)
        # theta = (1 - sqrt(q)) / (1 + sqrt(q)), q = 2*lam2*step
        wait("act", "dve", E_dv)
        sq = nc.scalar.activation(TN["sq"][g][:], TN["dv"][g][:], ACTF.Sqrt)
        E_sq = inc("act", sq)
        wait("dve", "act", E_sq)
        nc.vector.tensor_scalar(TN["onem"][g][:], TN["sq"][g][:], -1.0, 1.0,
                                ALU.mult, ALU.add)
        i = nc.vector.tensor_scalar(TN["onep"][g][:], TN["sq"][g][:], 1.0,
                                    None, ALU.add)
        dchain(i)
        i = nc.vector.reciprocal(TN["rop"][g][:], TN["onep"][g][:])
        dchain(i)
        nc.vector.tensor_tensor(TN["th"][g][:], TN["onem"][g][:],
                                TN["rop"][g][:], ALU.mult)
        # fv = step*(beta - lam1) + q*w_prev ; dm = diag(-2*step)
        wait("dve", "dma_bw", E_bw)
        i = nc.vector.tensor_scalar(fv[g][:], beta_g[g][:], lam1,
                                    TN["stp"][g][:], ALU.subtract, ALU.mult)
        dchain(i)
        nc.vector.scalar_tensor_tensor(fv[g][:], wprev_g[g][:],
                                       TN["dv"][g][:], fv[g][:],
                                       ALU.mult, ALU.add)
        i = nc.vector.tensor_scalar(dm[g][:], ident[0:GB, 0:GB],
                                    TN["m2a"][g][:], None, ALU.mult)
        E_fv = dchain(i)
        # fvT via PE transpose (identity) + ACT copy back to SBUF
        wait("pe", "dve", E_fv)
        tr = None
        for h in range(HALF):
            tr = nc.tensor.transpose(
                ptb[g][:, 2 * GB + h * GB:2 * GB + (h + 1) * GB],
                fv[g][:, h * 128:(h + 1) * 128],
                ident[0:GB, 0:GB])
        E_fvT = inc("pe", tr)
        wait("act", "pe", E_fvT)
        cp = None
        for h in range(HALF):
            cp = nc.scalar.copy(fvT[g][h][:, :],
                                ptb[g][:, 2 * GB + h * GB:2 * GB + (h + 1) * GB])
        E_fvTc = inc("act", cp)
        E_ptfree[g] = [("act", E_fvTc), ("dve", E_fv)]

    # ---------------- FISTA (two groups software-pipelined half a round
    # apart: group A's DVE chain runs while group B's matvec path is on
    # PE/ACT, and vice versa)
    def emit_matvec(g, ti):
        # PE: z -> zT (scaled by diag(-2*step)) via a REGULAR matmul (the
        # PE transpose datapath ignores the matrix values) ; ACT: -> SBUF;
        # PE: 130 matmuls ; ACT: yp -> staging ; PE: -> sample-major ysm
        wait("pe", "dve", E_z[g])
        for eng, tick in E_ptfree[g]:
            wait("pe", eng, tick)
        tr = None
        for h in range(HALF):
            tr = nc.tensor.matmul(
                ptb[g][:, h * GB:(h + 1) * GB],
                z[g][:, h * 128:(h + 1) * 128],
                dm[g][:, :], start=True, stop=True)
        E_pt = inc("pe", tr)
        wait("act", "pe", E_pt)
        cp = None
        for h in range(HALF):
            cp = nc.scalar.copy(zT[g][h][:, :],
                                ptb[g][:, h * GB:(h + 1) * GB])
        E_zT = inc("act", cp)
        E_ptfree[g] = [("act", E_zT)]
        wait("pe", "act", E_zT)
        mov = lambda hj, bb: zT[g][hj][:, bb:bb + 1]
        E_mm[g] = matvec(g, mov, with_fv=True, dma_gate=False,
                         yfree_tick=E_ycopy[g])
        wait("act", "pe", E_mm[g])
        stage = ybuf[g][ti % 2]
        wait("act", "pe", E_ybufread[g][ti % 2])
        cp = nc.scalar.copy(stage[:, :], yb[g][:, 0:HALF * GB])
        E_ycopy[g] = inc("act", cp)
        wait("pe", "act", E_ycopy[g])
        wait("pe", "dve", E_ysmfree[g])
        tr = None
        for hi in range(HALF):
            tr = nc.tensor.transpose(
                ysb[g][0:GB, hi * 128:(hi + 1) * 128],
                stage[:, hi * GB:(hi + 1) * GB],
                ident[:, :])
        E_ysm[g] = inc("pe", tr)
        E_ybufread[g][ti % 2] = E_ysm[g]

    def emit_chain(g, ti):
        wold = (wA if ti % 2 == 0 else wB)[g]
        wnew = (wB if ti % 2 == 0 else wA)[g]
        last = ti == T_FISTA - 1
        ysm = ysb[g][0:GB, 0:P]
        wait("dve", "pe", E_ysm[g])
        i = nc.vector.scalar_tensor_tensor(
            v[g][:], z[g][:], TN["ev"][g][:], ysm, ALU.mult, ALU.add,
            accum_out=(TN["sv"][g][:] if ti == 0 else None))
        E_ysmfree[g] = dchain(i)
        if ti == 0:
            i = nc.vector.tensor_scalar(TN["tau"][g][:], TN["sv"][g][:],
                                        1.0, 1.0 / P, ALU.subtract, ALU.mult)
            dchain(i)
            i = nc.vector.tensor_scalar(TN["tauc"][g][:], TN["tau"][g][:],
                                        MAX_W, None, ALU.add)
            dchain(i)
        # Newton step(s) on sum(clip(v - tau, 0, c)) = 1
        n_newton = NEWTON0 if ti == 0 else 1
        for it_n in range(n_newton):
            nc.vector.tensor_scalar(dum[g][:], v[g][:], TN["tau"][g][:],
                                    None, ALU.max, ALU.add,
                                    accum_out=TN["s1"][g][:])
            nc.vector.tensor_scalar(dum[g][:], v[g][:], TN["tauc"][g][:],
                                    None, ALU.max, ALU.add,
                                    accum_out=TN["s2"][g][:])
            i = nc.vector.tensor_scalar(dum[g][:], v[g][:],
                                        TN["tau"][g][:], None,
                                        ALU.is_gt, ALU.add,
                                        accum_out=TN["c1"][g][:])
            dchain(i)
            nc.vector.scalar_tensor_tensor(
                TN["phi"][g][:], TN["s1"][g][:], -(P * MAX_W - 1.0),
                TN["s2"][g][:], ALU.subtract, ALU.subtract)
            i = nc.vector.tensor_scalar(TN["cnt"][g][:], TN["c1"][g][:],
                                        1.0, None, ALU.add)
            dchain(i)
            i = nc.vector.reciprocal(TN["rc"][g][:], TN["cnt"][g][:])
            dchain(i)
            i = nc.vector.scalar_tensor_tensor(
                TN["tau"][g][:], TN["phi"][g][:], TN["rc"][g][:],
                TN["tau"][g][:], ALU.mult, ALU.add)
            dchain(i)
            if it_n < n_newton - 1:
                # next Newton step needs tauc now; otherwise it is
                # recomputed off the critical path after z below
                i = nc.vector.tensor_scalar(TN["tauc"][g][:],
                                            TN["tau"][g][:],
                                            MAX_W, None, ALU.add)
                dchain(i)
        # w_new = clip(v - tau, 0, MAX_W)
        i = nc.vector.tensor_scalar(t1[g][:], v[g][:], TN["tau"][g][:],
                                    0.0, ALU.subtract, ALU.max)
        dchain(i)
        if not last:
            nc.vector.tensor_scalar(wnew[:], t1[g][:], MAX_W, None,
                                    ALU.min)
            i = nc.vector.scalar_tensor_tensor(
                dw[g][:], t1[g][:], MAX_W, wold[:],
                ALU.min, ALU.subtract)
            dchain(i)
            zi = nc.vector.scalar_tensor_tensor(
                z[g][:], dw[g][:], TN["th"][g][:], wnew[:],
                ALU.mult, ALU.add)
            E_z[g] = inc("dve", zi)
            i = nc.vector.tensor_scalar(TN["tauc"][g][:], TN["tau"][g][:],
                                        MAX_W, None, ALU.add)
            dchain(i)
        else:
            # renormalize and stage the output
            i = nc.vector.tensor_scalar(wnew[:], t1[g][:], MAX_W, None,
                                        ALU.min, ALU.add,
                                        accum_out=TN["ssum"][g][:])
            dchain(i)
            i = nc.vector.tensor_scalar(TN["ssum"][g][:],
                                        TN["ssum"][g][:], EPS, None,
                                        ALU.add)
            dchain(i)
            i = nc.vector.reciprocal(TN["rs"][g][:], TN["ssum"][g][:])
            dchain(i)
            oi = nc.vector.tensor_scalar(outt[g][:], wnew[:],
                                         TN["rs"][g][:], None, ALU.mult)
            E_out[g] = inc("dve", oi)

    E_ysm = [0] * NGRP
    emit_matvec(0, 0)
    for ti in range(T_FISTA):
        emit_matvec(1, ti)
        emit_chain(0, ti)
        if ti + 1 < T_FISTA:
            emit_matvec(0, ti + 1)
        emit_chain(1, ti)

    # ---------------- store
    for g in range(NGRP):
        g0 = g * GB
        wait("sync", "dve", E_out[g])
        d = nc.sync.dma_start(out=out_d[g0:g0 + GB, :], in_=outt[g][:])
        d.then_inc(sems["dma_out"], 16)
    nc.sync.wait_ge(sems["dma_out"], 16 * NGRP)


def build(lam1, lam2):
    nc = bass.Bass("TRN2", target_bir_lowering=False, debug=False)
    sigma_d = nc.dram_tensor("sigma", [NB, P, P], F32, kind="ExternalInput")
    beta_d = nc.dram_tensor("beta", [NB, P], F32, kind="ExternalInput")
    wprev_d = nc.dram_tensor("w_prev", [NB, P], F32, kind="ExternalInput")
    out_d = nc.dram_tensor("out", [NB, P], F32, kind="ExternalOutput")
    with ExitStack() as ctx:
        _emit(ctx, nc, sigma_d.ap(), beta_d.ap(), wprev_d.ap(), out_d.ap(),
              lam1, lam2)
    return nc


def kernel(sigma, beta, w_prev, log_lambda1, log_lambda2):
    global LAST_RESULT
    sigma = np.ascontiguousarray(np.asarray(sigma, dtype=np.float32))
    beta = np.ascontiguousarray(np.asarray(beta, dtype=np.float32))
    w_prev = np.ascontiguousarray(np.asarray(w_prev, dtype=np.float32))
    lam1 = float(np.exp(np.float32(log_lambda1)))
    lam2 = float(np.exp(np.float32(log_lambda2)))

    nc = build(lam1, lam2)
    in_maps = []
    for c in range(N_CORES):
        s = slice(c * NB, (c + 1) * NB)
        in_maps.append({
            "sigma": sigma[s],
            "beta": beta[s],
            "w_prev": w_prev[s],
        })
    res = run_bass_kernel_spmd(nc, in_maps, list(range(N_CORES)), trace=TRACE)
    LAST_RESULT = res
    out = np.concatenate([res.results[c]["out"] for c in range(N_CORES)],
                         axis=0)
    return np.ascontiguousarray(out.astype(np.float32))


# revision 12
# speedup vs baseline: 8.1497x; 1.1324x over previous
"""Trainium2 Bass kernel for nn_DifferentiableRiskBudgeting.

Solves, per batch sample b:
    min_w  w' S_b w - beta_b' w + lam1*||w||_1 + lam2*||w - w_prev||^2
    s.t.   sum w = 1, 0 <= w <= MAX_W
then clamps + renormalizes — matching the reference's converged
projected-gradient solution. FISTA (T=20) with a warm-started 1-step
Newton projection per iteration replaces the reference's 250 PGD
iterations; validated in fp32 numpy against the reference to rel err
2.6e-3 (gate is 2e-2).

Sharding: pure data parallel, batch 512 = 64 samples per core on 8
cores, processed as two pipelined groups of 32 so the DVE projection
of one group overlaps the PE matvec path of the other.

Key structure (per group of 32 samples):
  - Power iterations (2) run entirely asset-major with NO
    normalization (power iteration is scale-free): PE matvec ->
    PSUM -> ACT copy to SBUF double buffer which IS the next round's
    transposed moving operand. Zero DVE work. A stale Rayleigh
    quotient (z_{k-1}.y_k / z_{k-1}.z_{k-1}) avoids an extra matvec.
  - FISTA rounds: DVE computes sample-major [32,256]; the per-sample
    step scale (-2*step) is folded into the PE z-transpose by using
    diag(-2*step) instead of the identity as the transpose matrix;
    the constant per-sample vector fv = step*(beta-lam1)+q*w_prev is
    added inside PSUM by one identity-stationary matmul per i-half,
    so v = ev*z + (S zs + fv) takes ONE DVE op.
  - y returns to sample-major via ACT copy (PSUM->SBUF) + PE
    transpose (SBUF->PSUM), keeping the 32x32 DVE StreamTransposes
    of the previous design off the critical engine.
  - Capped-simplex projection: 1 warm-started damped-Newton step on
    sum(clip(v-tau,0,c))=1 (slope = #(v>tau)+1); tau0 from the
    unconstrained solution (sum v - 1)/P on round 0 (2 steps there).

Raw bass (no Tile): this container's walrus build only allows ~2 sync
commands per instruction, which Tile's scheduler exceeds at every
cross-engine join. With explicit semaphores every wait is a standalone
single-wait instruction; all semaphore values are static because the
schedule is fully unrolled. Same-engine dependent ops also need a
producer-inc + consumer-wait pair (engine pipelines do not interlock),
with ordering transitive through any later same-engine inc.
"""

import math
import numpy as np
from contextlib import ExitStack

import concourse.bass as bass
from concourse import mybir
from concourse.bass_utils import run_bass_kernel_spmd

F32 = mybir.dt.float32
F16 = mybir.dt.float16
ALU = mybir.AluOpType
ACTF = mybir.ActivationFunctionType

B, P = 512, 256
N_CORES = 8
NB = B // N_CORES            # samples per core
HALF = P // 128              # sigma row-halves (2)
GB = 32                      # pipeline group size
NGRP = NB // GB
MAX_W = 0.1
EPS = 1e-8

NPOW = 2                     # scale-free power iterations
T_FISTA = 20                 # FISTA iterations
NEWTON0 = 2                  # Newton steps on the first projection
SAFETY = 1.4                 # L overestimation factor
SIG_DMA_BATCH = 4            # samples per sigma DMA

# set by the test harness; ignored by graders
TRACE = False
LAST_RESULT = None


def _emit(ctx, nc, sigma_d, beta_d, wprev_d, out_d, lam1, lam2):
    def sbuf(name, shape):
        return ctx.enter_context(nc.sbuf_tensor(name, shape, F32))

    def psum(name):
        # full-bank tensors so PE writes and DVE reads of different
        # buffers can never share a PSUM bank (fatal on HW)
        return ctx.enter_context(nc.psum_tensor(name, [128, 512], F32))

    sem_names = ["pe", "act", "dve", "pool", "dma_bw", "dma_out"]
    nk = (NB + SIG_DMA_BATCH - 1) // SIG_DMA_BATCH
    sem_names += [f"dsig{k}" for k in range(nk)]
    sems = {e: ctx.enter_context(nc.semaphore(f"s_{e}")) for e in sem_names}
    ENG = {"pe": nc.tensor, "dve": nc.vector, "act": nc.scalar,
           "pool": nc.gpsimd, "sync": nc.sync}
    ctr = {e: 0 for e in sems}
    last_wait = {e: {} for e in list(ENG)}

    def inc(ename, inst, n=1):
        ctr[ename] += n
        inst.then_inc(sems[ename], n)
        return ctr[ename]

    def wait(consumer, producer, value):
        if value is None or value <= 0:
            return
        lw = last_wait[consumer]
        if lw.get(producer, 0) >= value:
            return
        ENG[consumer].wait_ge(sems[producer], value)
        lw[producer] = value

    def dchain(inst):
        t = inc("dve", inst)
        wait("dve", "dve", t)
        return t

    # ---------------- tensors
    ident = sbuf("ident", [128, 128])
    nbatch = SIG_DMA_BATCH
    sig = [ctx.enter_context(
        nc.sbuf_tensor(f"sig{k}", [128, nbatch * HALF * P], F16))
        for k in range(nk)]

    def sig_ap(b, hj, hi):
        k, m = divmod(b, nbatch)
        c0 = (m * HALF + hj) * P + hi * 128
        return sig[k][:, c0:c0 + 128]

    def gt(name, shape):
        return [sbuf(f"{name}{g}", shape) for g in range(NGRP)]

    def gt16(name, shape):
        return [ctx.enter_context(nc.sbuf_tensor(f"{name}{g}", shape, F16))
                for g in range(NGRP)]

    z = gt16("z", [GB, P])
    v = gt16("v", [GB, P])
    t1 = gt16("t1", [GB, P])
    wA = gt16("wA", [GB, P])
    wB = gt16("wB", [GB, P])
    dw = gt16("dw", [GB, P])
    fv = gt("fv", [GB, P])
    beta_g = gt("beta", [GB, P])
    wprev_g = gt("wprev", [GB, P])
    outt = gt("outt", [GB, P])
    dum = gt16("dum", [GB, P])       # elementwise discard for accum ops
    zsm_sb = gt("zsm", [GB, P])      # sample-major z for the Rayleigh
    zTc = ctx.enter_context(nc.sbuf_tensor("zTc", [128, GB], F16))
    ybuf = [[ctx.enter_context(
        nc.sbuf_tensor(f"ybuf{g}_{p}", [128, HALF * GB], F16))
        for p in range(2)] for g in range(NGRP)]
    ystg = [[sbuf(f"ystg{g}_{p}", [128, HALF * GB]) for p in range(2)]
            for g in range(NGRP)]
    zT = [[ctx.enter_context(nc.sbuf_tensor(f"zT{g}_{h}", [128, GB], F16))
           for h in range(HALF)] for g in range(NGRP)]
    fvT = [[sbuf(f"fvT{g}_{h}", [128, GB]) for h in range(HALF)]
           for g in range(NGRP)]
    dm = [ctx.enter_context(nc.sbuf_tensor(f"dm{g}", [GB, GB], F16))
          for g in range(NGRP)]     # diag(-2*step) transpose matrix
    tiny_names = ("tau tauc s1 s2 c1 phi cnt rc num den rden lmax Lt stp "
                  "m2a dv ev sq onem onep rop th sv ssum rs")
    TN = {}
    for name in tiny_names.split():
        TN[name] = gt(name, [GB, 1])

    ptb = [psum(f"pt{g}") for g in range(NGRP)]    # z/fv transposes + zsm
    yb = [psum(f"y{g}") for g in range(NGRP)]      # matvec accumulator
    ysb = [psum(f"ys{g}") for g in range(NGRP)]    # sample-major y

    # ---------------- events (per group)
    E_z = [0] * NGRP            # dve: z ready for transpose
    E_ptfree = [[] for _ in range(NGRP)]   # [(engine, tick)]: pt consumed
    E_ycopy = [0] * NGRP        # act: yp copied out (ybank free)
    E_ysmfree = [0] * NGRP      # dve: ysm consumed
    E_ybufread = [[0, 0] for _ in range(NGRP)]  # pe: ybuf[parity] read
    E_out = [0] * NGRP

    # ---------------- preamble
    mz = nc.vector.memset(ident[:], 0.0)
    E_identz = inc("dve", mz)
    wait("pool", "dve", E_identz)
    af = nc.gpsimd.affine_select(
        out=ident[:], in_=ident[:], compare_op=ALU.not_equal, fill=1.0,
        base=0, pattern=[[-1, 128]], channel_multiplier=1)
    E_ident = inc("pool", af)

    for k in range(nk):
        kn = min(nbatch, NB - k * nbatch)
        srca = sigma_d[k * nbatch:k * nbatch + kn].rearrange(
            "b (h p) j -> p b h j", p=128)
        dst = sig[k][:].rearrange("p (b h j) -> p b h j", b=kn, h=HALF)
        d = nc.sync.dma_start(out=dst, in_=srca)
        d.then_inc(sems[f"dsig{k}"], 16)
    for g in range(NGRP):
        g0 = g * GB
        d = nc.sync.dma_start(out=beta_g[g][:], in_=beta_d[g0:g0 + GB, :])
        d.then_inc(sems["dma_bw"], 16)
        d = nc.sync.dma_start(out=wprev_g[g][:], in_=wprev_d[g0:g0 + GB, :])
        d.then_inc(sems["dma_bw"], 16)
    E_bw = 32 * NGRP

    nc.vector.memset(zTc[:], 1.0)
    for g in range(NGRP):
        nc.vector.memset(z[g][:], 1.0 / P)
        m = nc.vector.memset(wA[g][:], 1.0 / P)
        E_z[g] = inc("dve", m)
    E_zTc = E_z[NGRP - 1]

    # ---------------- helpers
    def matvec(g, moving, with_fv, dma_gate, yfree_tick):
        """PE: 128 (+2) matmuls accumulating y[i, hi*GB+s] into yb[g]."""
        wait("pe", "act", yfree_tick)
        if with_fv:
            # start=True clears the whole bank's has_written bits, so only
            # the first block may carry it; the second overwrites (hw=0).
            for hi in range(HALF):
                nc.tensor.matmul(yb[g][:, hi * GB:(hi + 1) * GB],
                                 ident[:, :], fvT[g][hi][:, :],
                                 start=(hi == 0), stop=False)
        g0 = g * GB
        mm = None
        for bb in range(GB):
            b = g0 + bb
            if dma_gate:
                wait("pe", f"dsig{b // nbatch}", 16)
            for hi in range(HALF):
                for hj in range(HALF):
                    mm = nc.tensor.matmul(
                        yb[g][:, hi * GB + bb:hi * GB + bb + 1],
                        sig_ap(b, hj, hi),
                        moving(hj, bb),
                        start=(hj == 0 and not with_fv),
                        stop=(hj == HALF - 1))
        return inc("pe", mm)

    # ---------------- power iterations (asset-major, no normalization)
    E_mm = [0] * NGRP
    for r in range(NPOW):
        for g in range(NGRP):
            if r == 0:
                wait("pe", "dve", E_zTc)
                mov = lambda hj, bb: zTc[:, bb:bb + 1]
            else:
                prev = ybuf[g][(r - 1) % 2]
                mov = (lambda pv: lambda hj, bb:
                       pv[:, hj * GB + bb:hj * GB + bb + 1])(prev)
                wait("pe", "act", E_ycopy[g])
            E_mm[g] = matvec(g, mov, with_fv=False, dma_gate=(r == 0),
                             yfree_tick=E_ycopy[g])
        for g in range(NGRP):
            wait("act", "pe", E_mm[g])
            if r < NPOW - 1:
                cp = nc.scalar.copy(ybuf[g][r % 2][:, :],
                                    yb[g][:, 0:HALF * GB])
            if r == NPOW - 2:
                cp = nc.scalar.copy(ystg[g][0][:, :], yb[g][:, 0:HALF * GB])
            if r == NPOW - 1:
                cp = nc.scalar.copy(ystg[g][1][:, :], yb[g][:, 0:HALF * GB])
            E_ycopy[g] = inc("act", cp)

    # ---------------- Rayleigh quotient + FISTA coefficients
    E_ray = [0] * NGRP
    for g in range(NGRP):
        wait("pe", "act", E_ycopy[g])
        wait("pe", "pool", E_ident)
        tr = None
        for hi in range(HALF):
            # zsm: matvec input of the last power round
            tr = nc.tensor.transpose(
                ptb[g][0:GB, hi * 128:(hi + 1) * 128],
                ystg[g][0][:, hi * GB:(hi + 1) * GB],
                ident[:, :])
        for hi in range(HALF):
            tr = nc.tensor.transpose(
                ysb[g][0:GB, hi * 128:(hi + 1) * 128],
                ystg[g][1][:, hi * GB:(hi + 1) * GB],
                ident[:, :])
        E_ray[g] = inc("pe", tr)
        E_ybufread[g][0] = E_ybufread[g][1] = E_ray[g]

    E_zsm = [0] * NGRP
    for g in range(NGRP):
        # DVE cannot read two PSUM banks in one op: stage zsm via ACT
        wait("act", "pe", E_ray[g])
        cp = nc.scalar.copy(zsm_sb[g][:], ptb[g][0:GB, 0:P])
        E_zsm[g] = inc("act", cp)

    for g in range(NGRP):
        ysm = ysb[g][0:GB, 0:P]
        wait("dve", "act", E_zsm[g])
        wait("dve", "pe", E_ray[g])
        nc.vector.scalar_tensor_tensor(dum[g][:], zsm_sb[g][:], 1.0, ysm,
                                       ALU.mult, ALU.mult,
                                       accum_out=TN["num"][g][:])
        i = nc.vector.scalar_tensor_tensor(dum[g][:], zsm_sb[g][:], 1.0,
                                           zsm_sb[g][:],
                                           ALU.mult, ALU.mult,
                                           accum_out=TN["den"][g][:])
        E_ysmfree[g] = dchain(i)

    for g in range(NGRP):
        i = nc.vector.tensor_scalar(TN["den"][g][:], TN["den"][g][:], EPS,
                                    None, ALU.add)
        dchain(i)
        i = nc.vector.reciprocal(TN["rden"][g][:], TN["den"][g][:])
        dchain(i)
        i = nc.vector.tensor_tensor(TN["lmax"][g][:], TN["num"][g][:],
                                    TN["rden"][g][:], ALU.mult)
        dchain(i)
        i = nc.vector.tensor_scalar(TN["Lt"][g][:], TN["lmax"][g][:],
                                    2.0 * SAFETY, SAFETY * 2.0 * lam2,
                                    ALU.mult, ALU.add)
        dchain(i)
        i = nc.vector.reciprocal(TN["stp"][g][:], TN["Lt"][g][:])
        dchain(i)
        nc.vector.tensor_scalar(TN["m2a"][g][:], TN["stp"][g][:], -2.0, None,
                                ALU.mult)
        dvi = nc.vector.tensor_scalar(TN["dv"][g][:], TN["stp"][g][:],
                                      2.0 * lam2, None, ALU.mult)
        E_dv = dchain(dvi)
        nc.vector.tensor_scalar(TN["ev"][g][:], TN["dv"][g][:], -1.0, 1.0,
                                ALU.mult, ALU.add)
        # theta = (1 - sqrt(q)) / (1 + sqrt(q)), q = 2*lam2*step
        wait("act", "dve", E_dv)
        sq = nc.scalar.activation(TN["sq"][g][:], TN["dv"][g][:], ACTF.Sqrt)
        E_sq = inc("act", sq)
        wait("dve", "act", E_sq)
        nc.vector.tensor_scalar(TN["onem"][g][:], TN["sq"][g][:], -1.0, 1.0,
                                ALU.mult, ALU.add)
        i = nc.vector.tensor_scalar(TN["onep"][g][:], TN["sq"][g][:], 1.0,
                                    None, ALU.add)
        dchain(i)
        i = nc.vector.reciprocal(TN["rop"][g][:], TN["onep"][g][:])
        dchain(i)
        nc.vector.tensor_tensor(TN["th"][g][:], TN["onem"][g][:],
                                TN["rop"][g][:], ALU.mult)
        # fv = step*(beta - lam1) + q*w_prev ; dm = diag(-2*step)
        wait("dve", "dma_bw", E_bw)
        i = nc.vector.tensor_scalar(fv[g][:], beta_g[g][:], lam1,
                                    TN["stp"][g][:], ALU.subtract, ALU.mult)
        dchain(i)
        nc.vector.scalar_tensor_tensor(fv[g][:], wprev_g[g][:],
                                       TN["dv"][g][:], fv[g][:],
                                       ALU.mult, ALU.add)
        i = nc.vector.tensor_scalar(dm[g][:], ident[0:GB, 0:GB],
                                    TN["m2a"][g][:], None, ALU.mult)
        E_fv = dchain(i)
        # fvT via PE transpose (identity) + ACT copy back to SBUF
        wait("pe", "dve", E_fv)
        tr = None
        for h in range(HALF):
            tr = nc.tensor.transpose(
                ptb[g][:, 2 * GB + h * GB:2 * GB + (h + 1) * GB],
                fv[g][:, h * 128:(h + 1) * 128],
                ident[0:GB, 0:GB])
        E_fvT = inc("pe", tr)
        wait("act", "pe", E_fvT)
        cp = None
        for h in range(HALF):
            cp = nc.scalar.copy(fvT[g][h][:, :],
                                ptb[g][:, 2 * GB + h * GB:2 * GB + (h + 1) * GB])
        E_fvTc = inc("act", cp)
        E_ptfree[g] = [("act", E_fvTc), ("dve", E_fv)]

    # ---------------- FISTA (two groups software-pipelined half a round
    # apart: group A's DVE chain runs while group B's matvec path is on
    # PE/ACT, and vice versa)
    def emit_matvec(g, ti):
        # PE: z -> zT (scaled by diag(-2*step)) via a REGULAR matmul (the
        # PE transpose datapath ignores the matrix values) ; ACT: -> SBUF;
        # PE: 130 matmuls ; ACT: yp -> staging ; PE: -> sample-major ysm
        wait("pe", "dve", E_z[g])
        for eng, tick in E_ptfree[g]:
            wait("pe", eng, tick)
        tr = None
        for h in range(HALF):
            tr = nc.tensor.matmul(
                ptb[g][:, h * GB:(h + 1) * GB],
                z[g][:, h * 128:(h + 1) * 128],
                dm[g][:, :], start=True, stop=True)
        E_pt = inc("pe", tr)
        wait("act", "pe", E_pt)
        cp = None
        for h in range(HALF):
            cp = nc.scalar.copy(zT[g][h][:, :],
                                ptb[g][:, h * GB:(h + 1) * GB])
        E_zT = inc("act", cp)
        E_ptfree[g] = [("act", E_zT)]
        wait("pe", "act", E_zT)
        mov = lambda hj, bb: zT[g][hj][:, bb:bb + 1]
        E_mm[g] = matvec(g, mov, with_fv=True, dma_gate=False,
                         yfree_tick=E_ycopy[g])
        wait("act", "pe", E_mm[g])
        stage = ystg[g][ti % 2]
        wait("act", "pe", E_ybufread[g][ti % 2])
        cp = nc.scalar.copy(stage[:, :], yb[g][:, 0:HALF * GB])
        E_ycopy[g] = inc("act", cp)
        wait("pe", "act", E_ycopy[g])
        wait("pe", "dve", E_ysmfree[g])
        tr = None
        for hi in range(HALF):
            tr = nc.tensor.transpose(
                ysb[g][0:GB, hi * 128:(hi + 1) * 128],
                stage[:, hi * GB:(hi + 1) * GB],
                ident[:, :])
        E_ysm[g] = inc("pe", tr)
        E_ybufread[g][ti % 2] = E_ysm[g]

    def emit_chain(g, ti):
        wold = (wA if ti % 2 == 0 else wB)[g]
        wnew = (wB if ti % 2 == 0 else wA)[g]
        last = ti == T_FISTA - 1
        ysm = ysb[g][0:GB, 0:P]
        wait("dve", "pe", E_ysm[g])
        i = nc.vector.scalar_tensor_tensor(
            v[g][:], z[g][:], TN["ev"][g][:], ysm, ALU.mult, ALU.add,
            accum_out=(TN["sv"][g][:] if ti == 0 else None))
        E_ysmfree[g] = dchain(i)
        if ti == 0:
            i = nc.vector.tensor_scalar(TN["tau"][g][:], TN["sv"][g][:],
                                        1.0, 1.0 / P, ALU.subtract, ALU.mult)
            dchain(i)
            i = nc.vector.tensor_scalar(TN["tauc"][g][:], TN["tau"][g][:],
                                        MAX_W, None, ALU.add)
            dchain(i)
        # Newton step(s) on sum(clip(v - tau, 0, c)) = 1
        n_newton = NEWTON0 if ti == 0 else 1
        for it_n in range(n_newton):
            nc.vector.tensor_scalar(dum[g][:], v[g][:], TN["tau"][g][:],
                                    None, ALU.max, ALU.add,
                                    accum_out=TN["s1"][g][:])
            nc.vector.tensor_scalar(dum[g][:], v[g][:], TN["tauc"][g][:],
                                    None, ALU.max, ALU.add,
                                    accum_out=TN["s2"][g][:])
            i = nc.vector.tensor_scalar(dum[g][:], v[g][:],
                                        TN["tau"][g][:], None,
                                        ALU.is_gt, ALU.add,
                                        accum_out=TN["c1"][g][:])
            dchain(i)
            nc.vector.scalar_tensor_tensor(
                TN["phi"][g][:], TN["s1"][g][:], -(P * MAX_W - 1.0),
                TN["s2"][g][:], ALU.subtract, ALU.subtract)
            i = nc.vector.tensor_scalar(TN["cnt"][g][:], TN["c1"][g][:],
                                        1.0, None, ALU.add)
            dchain(i)
            i = nc.vector.reciprocal(TN["rc"][g][:], TN["cnt"][g][:])
            dchain(i)
            i = nc.vector.scalar_tensor_tensor(
                TN["tau"][g][:], TN["phi"][g][:], TN["rc"][g][:],
                TN["tau"][g][:], ALU.mult, ALU.add)
            dchain(i)
            if it_n < n_newton - 1:
                # next Newton step needs tauc now; otherwise it is
                # recomputed off the critical path after z below
                i = nc.vector.tensor_scalar(TN["tauc"][g][:],
                                            TN["tau"][g][:],
                                            MAX_W, None, ALU.add)
                dchain(i)
        # w_new = clip(v - tau, 0, MAX_W)
        i = nc.vector.tensor_scalar(t1[g][:], v[g][:], TN["tau"][g][:],
                                    0.0, ALU.subtract, ALU.max)
        dchain(i)
        if not last:
            nc.vector.tensor_scalar(wnew[:], t1[g][:], MAX_W, None,
                                    ALU.min)
            i = nc.vector.scalar_tensor_tensor(
                dw[g][:], t1[g][:], MAX_W, wold[:],
                ALU.min, ALU.subtract)
            dchain(i)
            zi = nc.vector.scalar_tensor_tensor(
                z[g][:], dw[g][:], TN["th"][g][:], wnew[:],
                ALU.mult, ALU.add)
            E_z[g] = inc("dve", zi)
            i = nc.vector.tensor_scalar(TN["tauc"][g][:], TN["tau"][g][:],
                                        MAX_W, None, ALU.add)
            dchain(i)
        else:
            # renormalize and stage the output
            i = nc.vector.tensor_scalar(wnew[:], t1[g][:], MAX_W, None,
                                        ALU.min, ALU.add,
                                        accum_out=TN["ssum"][g][:])
            dchain(i)
            i = nc.vector.tensor_scalar(TN["ssum"][g][:],
                                        TN["ssum"][g][:], EPS, None,
                                        ALU.add)
            dchain(i)
            i = nc.vector.reciprocal(TN["rs"][g][:], TN["ssum"][g][:])
            dchain(i)
            oi = nc.vector.tensor_scalar(outt[g][:], wnew[:],
                                         TN["rs"][g][:], None, ALU.mult)
            E_out[g] = inc("dve", oi)

    E_ysm = [0] * NGRP
    emit_matvec(0, 0)
    for ti in range(T_FISTA):
        emit_matvec(1, ti)
        emit_chain(0, ti)
        if ti + 1 < T_FISTA:
            emit_matvec(0, ti + 1)
        emit_chain(1, ti)

    # ---------------- store
    for g in range(NGRP):
        g0 = g * GB
        wait("sync", "dve", E_out[g])
        d = nc.sync.dma_start(out=out_d[g0:g0 + GB, :], in_=outt[g][:])
        d.then_inc(sems["dma_out"], 16)
    nc.sync.wait_ge(sems["dma_out"], 16 * NGRP)


def build(lam1, lam2):
    nc = bass.Bass("TRN2", target_bir_lowering=False, debug=False)
    sigma_d = nc.dram_tensor("sigma", [NB, P, P], F16, kind="ExternalInput")
    beta_d = nc.dram_tensor("beta", [NB, P], F32, kind="ExternalInput")
    wprev_d = nc.dram_tensor("w_prev", [NB, P], F32, kind="ExternalInput")
    out_d = nc.dram_tensor("out", [NB, P], F32, kind="ExternalOutput")
    with ExitStack() as ctx:
        _emit(ctx, nc, sigma_d.ap(), beta_d.ap(), wprev_d.ap(), out_d.ap(),
              lam1, lam2)
    return nc


def kernel(sigma, beta, w_prev, log_lambda1, log_lambda2):
    global LAST_RESULT
    sigma = np.ascontiguousarray(np.asarray(sigma, dtype=np.float32))
    beta = np.ascontiguousarray(np.asarray(beta, dtype=np.float32))
    w_prev = np.ascontiguousarray(np.asarray(w_prev, dtype=np.float32))
    lam1 = float(np.exp(np.float32(log_lambda1)))
    lam2 = float(np.exp(np.float32(log_lambda2)))

    nc = build(lam1, lam2)
    in_maps = []
    for c in range(N_CORES):
        s = slice(c * NB, (c + 1) * NB)
        in_maps.append({
            "sigma": np.ascontiguousarray(sigma[s].astype(np.float16)),
            "beta": beta[s],
            "w_prev": w_prev[s],
        })
    res = run_bass_kernel_spmd(nc, in_maps, list(range(N_CORES)), trace=TRACE)
    LAST_RESULT = res
    out = np.concatenate([res.results[c]["out"] for c in range(N_CORES)],
                         axis=0)
    return np.ascontiguousarray(out.astype(np.float32))


# revision 15
# speedup vs baseline: 8.3726x; 1.0273x over previous
"""Trainium2 Bass kernel for nn_DifferentiableRiskBudgeting.

Solves, per batch sample b:
    min_w  w' S_b w - beta_b' w + lam1*||w||_1 + lam2*||w - w_prev||^2
    s.t.   sum w = 1, 0 <= w <= MAX_W
then clamps + renormalizes — matching the reference's converged
projected-gradient solution. FISTA (T=20) with a warm-started 1-step
Newton projection per iteration replaces the reference's 250 PGD
iterations; validated in fp32 numpy against the reference to rel err
2.6e-3 (gate is 2e-2).

Sharding: pure data parallel, batch 512 = 64 samples per core on 8
cores, processed as two pipelined groups of 32 so the DVE projection
of one group overlaps the PE matvec path of the other.

Key structure (per group of 32 samples):
  - Power iterations (2) run entirely asset-major with NO
    normalization (power iteration is scale-free): PE matvec ->
    PSUM -> ACT copy to SBUF double buffer which IS the next round's
    transposed moving operand. Zero DVE work. A stale Rayleigh
    quotient (z_{k-1}.y_k / z_{k-1}.z_{k-1}) avoids an extra matvec.
  - FISTA rounds: DVE computes sample-major [32,256]; the per-sample
    step scale (-2*step) is folded into the PE z-transpose by using
    diag(-2*step) instead of the identity as the transpose matrix;
    the constant per-sample vector fv = step*(beta-lam1)+q*w_prev is
    added inside PSUM by one identity-stationary matmul per i-half,
    so v = ev*z + (S zs + fv) takes ONE DVE op.
  - y returns to sample-major via ACT copy (PSUM->SBUF) + PE
    transpose (SBUF->PSUM), keeping the 32x32 DVE StreamTransposes
    of the previous design off the critical engine.
  - Capped-simplex projection: 1 warm-started damped-Newton step on
    sum(clip(v-tau,0,c))=1 (slope = #(v>tau)+1); tau0 from the
    unconstrained solution (sum v - 1)/P on round 0 (2 steps there).

Raw bass (no Tile): this container's walrus build only allows ~2 sync
commands per instruction, which Tile's scheduler exceeds at every
cross-engine join. With explicit semaphores every wait is a standalone
single-wait instruction; all semaphore values are static because the
schedule is fully unrolled. Same-engine dependent ops also need a
producer-inc + consumer-wait pair (engine pipelines do not interlock),
with ordering transitive through any later same-engine inc.
"""

import math
import numpy as np
from contextlib import ExitStack

import concourse.bass as bass
from concourse import mybir
from concourse.bass_utils import run_bass_kernel_spmd

F32 = mybir.dt.float32
F16 = mybir.dt.float16
ALU = mybir.AluOpType
ACTF = mybir.ActivationFunctionType

B, P = 512, 256
N_CORES = 8
NB = B // N_CORES            # samples per core
HALF = P // 128              # sigma row-halves (2)
GB = 32                      # pipeline group size
NGRP = NB // GB
MAX_W = 0.1
EPS = 1e-8

NPOW = 2                     # scale-free power iterations
T_FISTA = 20                 # FISTA iterations
NEWTON0 = 2                  # Newton steps on the first projection
SAFETY = 1.4                 # L overestimation factor
SIG_DMA_BATCH = 4            # samples per sigma DMA

# set by the test harness; ignored by graders
TRACE = False
LAST_RESULT = None


def _emit(ctx, nc, sigma_d, beta_d, wprev_d, out_d, lam1, lam2):
    def sbuf(name, shape):
        return ctx.enter_context(nc.sbuf_tensor(name, shape, F32))

    def psum(name):
        # full-bank tensors so PE writes and DVE reads of different
        # buffers can never share a PSUM bank (fatal on HW)
        return ctx.enter_context(nc.psum_tensor(name, [128, 512], F32))

    sem_names = ["pe", "act", "dve", "pool", "dma_bw", "dma_out"]
    nk = (NB + SIG_DMA_BATCH - 1) // SIG_DMA_BATCH
    sem_names += [f"dsig{k}" for k in range(nk)]
    sems = {e: ctx.enter_context(nc.semaphore(f"s_{e}")) for e in sem_names}
    ENG = {"pe": nc.tensor, "dve": nc.vector, "act": nc.scalar,
           "pool": nc.gpsimd, "sync": nc.sync}
    ctr = {e: 0 for e in sems}
    last_wait = {e: {} for e in list(ENG)}

    def inc(ename, inst, n=1):
        ctr[ename] += n
        inst.then_inc(sems[ename], n)
        return ctr[ename]

    def wait(consumer, producer, value):
        if value is None or value <= 0:
            return
        lw = last_wait[consumer]
        if lw.get(producer, 0) >= value:
            return
        ENG[consumer].wait_ge(sems[producer], value)
        lw[producer] = value

    def dchain(inst):
        t = inc("dve", inst)
        wait("dve", "dve", t)
        return t

    # ---------------- tensors
    ident = sbuf("ident", [128, 128])
    nbatch = SIG_DMA_BATCH
    sig = [ctx.enter_context(
        nc.sbuf_tensor(f"sig{k}", [128, nbatch * HALF * P], F16))
        for k in range(nk)]

    def sig_ap(b, hj, hi):
        k, m = divmod(b, nbatch)
        c0 = (m * HALF + hj) * P + hi * 128
        return sig[k][:, c0:c0 + 128]

    def gt(name, shape):
        return [sbuf(f"{name}{g}", shape) for g in range(NGRP)]

    def gt16(name, shape):
        return [ctx.enter_context(nc.sbuf_tensor(f"{name}{g}", shape, F16))
                for g in range(NGRP)]

    z = gt16("z", [GB, P])
    v = gt16("v", [GB, P])
    t1 = gt16("t1", [GB, P])
    wA = gt16("wA", [GB, P])
    wB = gt16("wB", [GB, P])
    dw = gt16("dw", [GB, P])
    fv = gt("fv", [GB, P])
    beta_g = gt("beta", [GB, P])
    wprev_g = gt("wprev", [GB, P])
    outt = gt("outt", [GB, P])
    dum = gt16("dum", [GB, P])       # elementwise discard for accum ops
    u0 = gt16("u0", [GB, P])         # v - tau_old (pre-subtracted)
    thwold = gt16("thwold", [GB, P])  # th * w_old (computed off-path)
    zsm_sb = gt("zsm", [GB, P])      # sample-major z for the Rayleigh
    zTc = ctx.enter_context(nc.sbuf_tensor("zTc", [128, GB], F16))
    ybuf = [[ctx.enter_context(
        nc.sbuf_tensor(f"ybuf{g}_{p}", [128, HALF * GB], F16))
        for p in range(2)] for g in range(NGRP)]
    ystg = [[sbuf(f"ystg{g}_{p}", [128, HALF * GB]) for p in range(2)]
            for g in range(NGRP)]
    zT = [[ctx.enter_context(nc.sbuf_tensor(f"zT{g}_{h}", [128, GB], F16))
           for h in range(HALF)] for g in range(NGRP)]
    fvT = [[sbuf(f"fvT{g}_{h}", [128, GB]) for h in range(HALF)]
           for g in range(NGRP)]
    dm = [ctx.enter_context(nc.sbuf_tensor(f"dm{g}", [GB, GB], F16))
          for g in range(NGRP)]     # diag(-2*step) transpose matrix
    tiny_names = ("tau tauc s1 s2 c1 phi cnt rc num den rden lmax Lt stp "
                  "m2a dv ev sq onem onep rop th sv ssum rs opth thr dlt")
    TN = {}
    for name in tiny_names.split():
        TN[name] = gt(name, [GB, 1])

    ptb = [psum(f"pt{g}") for g in range(NGRP)]    # z/fv transposes + zsm
    yb = [psum(f"y{g}") for g in range(NGRP)]      # matvec accumulator
    ysb = [psum(f"ys{g}") for g in range(NGRP)]    # sample-major y

    # ---------------- events (per group)
    E_z = [0] * NGRP            # dve: z ready for transpose
    E_ptfree = [[] for _ in range(NGRP)]   # [(engine, tick)]: pt consumed
    E_ycopy = [0] * NGRP        # act: yp copied out (ybank free)
    E_ysmfree = [0] * NGRP      # dve: ysm consumed
    E_ybufread = [[0, 0] for _ in range(NGRP)]  # pe: ybuf[parity] read
    E_out = [0] * NGRP

    # ---------------- preamble
    mz = nc.vector.memset(ident[:], 0.0)
    E_identz = inc("dve", mz)
    wait("pool", "dve", E_identz)
    af = nc.gpsimd.affine_select(
        out=ident[:], in_=ident[:], compare_op=ALU.not_equal, fill=1.0,
        base=0, pattern=[[-1, 128]], channel_multiplier=1)
    E_ident = inc("pool", af)

    for k in range(nk):
        kn = min(nbatch, NB - k * nbatch)
        srca = sigma_d[k * nbatch:k * nbatch + kn].rearrange(
            "b (h p) j -> p b h j", p=128)
        dst = sig[k][:].rearrange("p (b h j) -> p b h j", b=kn, h=HALF)
        d = nc.sync.dma_start(out=dst, in_=srca)
        d.then_inc(sems[f"dsig{k}"], 16)
    for g in range(NGRP):
        g0 = g * GB
        d = nc.sync.dma_start(out=beta_g[g][:], in_=beta_d[g0:g0 + GB, :])
        d.then_inc(sems["dma_bw"], 16)
        d = nc.sync.dma_start(out=wprev_g[g][:], in_=wprev_d[g0:g0 + GB, :])
        d.then_inc(sems["dma_bw"], 16)
    E_bw = 32 * NGRP

    nc.vector.memset(zTc[:], 1.0)
    for g in range(NGRP):
        nc.vector.memset(z[g][:], 1.0 / P)
        m = nc.vector.memset(wA[g][:], 1.0 / P)
        E_z[g] = inc("dve", m)
    E_zTc = E_z[NGRP - 1]

    # ---------------- helpers
    def matvec(g, moving, with_fv, dma_gate, yfree_tick):
        """PE: 128 (+2) matmuls accumulating y[i, hi*GB+s] into yb[g]."""
        wait("pe", "act", yfree_tick)
        if with_fv:
            # start=True clears the whole bank's has_written bits, so only
            # the first block may carry it; the second overwrites (hw=0).
            for hi in range(HALF):
                nc.tensor.matmul(yb[g][:, hi * GB:(hi + 1) * GB],
                                 ident[:, :], fvT[g][hi][:, :],
                                 start=(hi == 0), stop=False)
        g0 = g * GB
        mm = None
        for bb in range(GB):
            b = g0 + bb
            if dma_gate:
                wait("pe", f"dsig{b // nbatch}", 16)
            for hi in range(HALF):
                for hj in range(HALF):
                    mm = nc.tensor.matmul(
                        yb[g][:, hi * GB + bb:hi * GB + bb + 1],
                        sig_ap(b, hj, hi),
                        moving(hj, bb),
                        start=(hj == 0 and not with_fv),
                        stop=(hj == HALF - 1))
        return inc("pe", mm)

    # ---------------- power iterations (asset-major, no normalization)
    E_mm = [0] * NGRP
    for r in range(NPOW):
        for g in range(NGRP):
            if r == 0:
                wait("pe", "dve", E_zTc)
                mov = lambda hj, bb: zTc[:, bb:bb + 1]
            else:
                prev = ybuf[g][(r - 1) % 2]
                mov = (lambda pv: lambda hj, bb:
                       pv[:, hj * GB + bb:hj * GB + bb + 1])(prev)
                wait("pe", "act", E_ycopy[g])
            E_mm[g] = matvec(g, mov, with_fv=False, dma_gate=(r == 0),
                             yfree_tick=E_ycopy[g])
        for g in range(NGRP):
            wait("act", "pe", E_mm[g])
            if r < NPOW - 1:
                cp = nc.scalar.copy(ybuf[g][r % 2][:, :],
                                    yb[g][:, 0:HALF * GB])
            if r == NPOW - 2:
                cp = nc.scalar.copy(ystg[g][0][:, :], yb[g][:, 0:HALF * GB])
            if r == NPOW - 1:
                cp = nc.scalar.copy(ystg[g][1][:, :], yb[g][:, 0:HALF * GB])
            E_ycopy[g] = inc("act", cp)

    # ---------------- Rayleigh quotient + FISTA coefficients
    E_ray = [0] * NGRP
    for g in range(NGRP):
        wait("pe", "act", E_ycopy[g])
        wait("pe", "pool", E_ident)
        tr = None
        for hi in range(HALF):
            # zsm: matvec input of the last power round
            tr = nc.tensor.transpose(
                ptb[g][0:GB, hi * 128:(hi + 1) * 128],
                ystg[g][0][:, hi * GB:(hi + 1) * GB],
                ident[:, :])
        for hi in range(HALF):
            tr = nc.tensor.transpose(
                ysb[g][0:GB, hi * 128:(hi + 1) * 128],
                ystg[g][1][:, hi * GB:(hi + 1) * GB],
                ident[:, :])
        E_ray[g] = inc("pe", tr)
        E_ybufread[g][0] = E_ybufread[g][1] = E_ray[g]

    E_zsm = [0] * NGRP
    for g in range(NGRP):
        # DVE cannot read two PSUM banks in one op: stage zsm via ACT
        wait("act", "pe", E_ray[g])
        cp = nc.scalar.copy(zsm_sb[g][:], ptb[g][0:GB, 0:P])
        E_zsm[g] = inc("act", cp)

    for g in range(NGRP):
        ysm = ysb[g][0:GB, 0:P]
        wait("dve", "act", E_zsm[g])
        wait("dve", "pe", E_ray[g])
        nc.vector.scalar_tensor_tensor(dum[g][:], zsm_sb[g][:], 1.0, ysm,
                                       ALU.mult, ALU.mult,
                                       accum_out=TN["num"][g][:])
        i = nc.vector.scalar_tensor_tensor(dum[g][:], zsm_sb[g][:], 1.0,
                                           zsm_sb[g][:],
                                           ALU.mult, ALU.mult,
                                           accum_out=TN["den"][g][:])
        E_ysmfree[g] = dchain(i)

    for g in range(NGRP):
        i = nc.vector.tensor_scalar(TN["den"][g][:], TN["den"][g][:], EPS,
                                    None, ALU.add)
        dchain(i)
        i = nc.vector.reciprocal(TN["rden"][g][:], TN["den"][g][:])
        dchain(i)
        i = nc.vector.tensor_tensor(TN["lmax"][g][:], TN["num"][g][:],
                                    TN["rden"][g][:], ALU.mult)
        dchain(i)
        i = nc.vector.tensor_scalar(TN["Lt"][g][:], TN["lmax"][g][:],
                                    2.0 * SAFETY, SAFETY * 2.0 * lam2,
                                    ALU.mult, ALU.add)
        dchain(i)
        i = nc.vector.reciprocal(TN["stp"][g][:], TN["Lt"][g][:])
        dchain(i)
        nc.vector.tensor_scalar(TN["m2a"][g][:], TN["stp"][g][:], -2.0, None,
                                ALU.mult)
        dvi = nc.vector.tensor_scalar(TN["dv"][g][:], TN["stp"][g][:],
                                      2.0 * lam2, None, ALU.mult)
        E_dv = dchain(dvi)
        nc.vector.tensor_scalar(TN["ev"][g][:], TN["dv"][g][:], -1.0, 1.0,
                                ALU.mult, ALU.add)
        # theta = (1 - sqrt(q)) / (1 + sqrt(q)), q = 2*lam2*step
        wait("act", "dve", E_dv)
        sq = nc.scalar.activation(TN["sq"][g][:], TN["dv"][g][:], ACTF.Sqrt)
        E_sq = inc("act", sq)
        wait("dve", "act", E_sq)
        nc.vector.tensor_scalar(TN["onem"][g][:], TN["sq"][g][:], -1.0, 1.0,
                                ALU.mult, ALU.add)
        i = nc.vector.tensor_scalar(TN["onep"][g][:], TN["sq"][g][:], 1.0,
                                    None, ALU.add)
        dchain(i)
        i = nc.vector.reciprocal(TN["rop"][g][:], TN["onep"][g][:])
        dchain(i)
        i = nc.vector.tensor_tensor(TN["th"][g][:], TN["onem"][g][:],
                                    TN["rop"][g][:], ALU.mult)
        dchain(i)
        i = nc.vector.tensor_scalar(TN["opth"][g][:], TN["th"][g][:], 1.0,
                                    None, ALU.add)
        dchain(i)
        i = nc.vector.reciprocal(TN["rden"][g][:], TN["opth"][g][:])
        dchain(i)
        nc.vector.tensor_tensor(TN["thr"][g][:], TN["th"][g][:],
                                TN["rden"][g][:], ALU.mult)
        # fv = step*(beta - lam1) + q*w_prev ; dm = diag(-2*step)
        wait("dve", "dma_bw", E_bw)
        i = nc.vector.tensor_scalar(fv[g][:], beta_g[g][:], lam1,
                                    TN["stp"][g][:], ALU.subtract, ALU.mult)
        dchain(i)
        nc.vector.scalar_tensor_tensor(fv[g][:], wprev_g[g][:],
                                       TN["dv"][g][:], fv[g][:],
                                       ALU.mult, ALU.add)
        i = nc.vector.tensor_scalar(dm[g][:], ident[0:GB, 0:GB],
                                    TN["m2a"][g][:], None, ALU.mult)
        E_fv = dchain(i)
        # fvT via PE transpose (identity) + ACT copy back to SBUF
        wait("pe", "dve", E_fv)
        tr = None
        for h in range(HALF):
            tr = nc.tensor.transpose(
                ptb[g][:, 2 * GB + h * GB:2 * GB + (h + 1) * GB],
                fv[g][:, h * 128:(h + 1) * 128],
                ident[0:GB, 0:GB])
        E_fvT = inc("pe", tr)
        wait("act", "pe", E_fvT)
        cp = None
        for h in range(HALF):
            cp = nc.scalar.copy(fvT[g][h][:, :],
                                ptb[g][:, 2 * GB + h * GB:2 * GB + (h + 1) * GB])
        E_fvTc = inc("act", cp)
        E_ptfree[g] = [("act", E_fvTc), ("dve", E_fv)]

    # ---------------- FISTA (two groups software-pipelined half a round
    # apart: group A's DVE chain runs while group B's matvec path is on
    # PE/ACT, and vice versa)
    def emit_matvec(g, ti):
        # PE: z -> zT (scaled by diag(-2*step)) via a REGULAR matmul (the
        # PE transpose datapath ignores the matrix values) ; ACT: -> SBUF;
        # PE: 130 matmuls ; ACT: yp -> staging ; PE: -> sample-major ysm
        wait("pe", "dve", E_z[g])
        for eng, tick in E_ptfree[g]:
            wait("pe", eng, tick)
        tr = None
        for h in range(HALF):
            tr = nc.tensor.matmul(
                ptb[g][:, h * GB:(h + 1) * GB],
                z[g][:, h * 128:(h + 1) * 128],
                dm[g][:, :], start=True, stop=True)
        E_pt = inc("pe", tr)
        wait("act", "pe", E_pt)
        cp = None
        for h in range(HALF):
            cp = nc.scalar.copy(zT[g][h][:, :],
                                ptb[g][:, h * GB:(h + 1) * GB])
        E_zT = inc("act", cp)
        E_ptfree[g] = [("act", E_zT)]
        wait("pe", "act", E_zT)
        mov = lambda hj, bb: zT[g][hj][:, bb:bb + 1]
        E_mm[g] = matvec(g, mov, with_fv=True, dma_gate=False,
                         yfree_tick=E_ycopy[g])
        wait("act", "pe", E_mm[g])
        stage = ystg[g][ti % 2]
        wait("act", "pe", E_ybufread[g][ti % 2])
        cp = nc.scalar.copy(stage[:, :], yb[g][:, 0:HALF * GB])
        E_ycopy[g] = inc("act", cp)
        wait("pe", "act", E_ycopy[g])
        wait("pe", "dve", E_ysmfree[g])
        tr = None
        for hi in range(HALF):
            tr = nc.tensor.transpose(
                ysb[g][0:GB, hi * 128:(hi + 1) * 128],
                stage[:, hi * GB:(hi + 1) * GB],
                ident[:, :])
        E_ysm[g] = inc("pe", tr)
        E_ybufread[g][ti % 2] = E_ysm[g]

    def emit_chain(g, ti):
        wold = (wA if ti % 2 == 0 else wB)[g]
        wnew = (wB if ti % 2 == 0 else wA)[g]
        last = ti == T_FISTA - 1
        ysm = ysb[g][0:GB, 0:P]
        wait("dve", "pe", E_ysm[g])
        i = nc.vector.scalar_tensor_tensor(
            v[g][:], z[g][:], TN["ev"][g][:], ysm, ALU.mult, ALU.add,
            accum_out=(TN["sv"][g][:] if ti == 0 else None))
        E_ysmfree[g] = dchain(i)
        if ti == 0:
            # cold start: tau0 from the unconstrained solution, then
            # NEWTON0 full Newton steps (fresh slope each)
            i = nc.vector.tensor_scalar(TN["tau"][g][:], TN["sv"][g][:],
                                        1.0, 1.0 / P, ALU.subtract, ALU.mult)
            dchain(i)
            i = nc.vector.tensor_scalar(TN["tauc"][g][:], TN["tau"][g][:],
                                        MAX_W, None, ALU.add)
            dchain(i)
            for it_n in range(NEWTON0):
                nc.vector.tensor_scalar(dum[g][:], v[g][:], TN["tau"][g][:],
                                        None, ALU.max, ALU.add,
                                        accum_out=TN["s1"][g][:])
                nc.vector.tensor_scalar(dum[g][:], v[g][:], TN["tauc"][g][:],
                                        None, ALU.max, ALU.add,
                                        accum_out=TN["s2"][g][:])
                i = nc.vector.tensor_scalar(dum[g][:], v[g][:],
                                            TN["tau"][g][:], None,
                                            ALU.is_gt, ALU.add,
                                            accum_out=TN["c1"][g][:])
                dchain(i)
                nc.vector.scalar_tensor_tensor(
                    TN["phi"][g][:], TN["s1"][g][:], -(P * MAX_W - 1.0),
                    TN["s2"][g][:], ALU.subtract, ALU.subtract)
                i = nc.vector.tensor_scalar(TN["cnt"][g][:], TN["c1"][g][:],
                                            1.0, None, ALU.add)
                dchain(i)
                i = nc.vector.reciprocal(TN["rc"][g][:], TN["cnt"][g][:])
                dchain(i)
                i = nc.vector.tensor_scalar(TN["dlt"][g][:], TN["phi"][g][:],
                                            TN["rc"][g][:], None, ALU.mult)
                dchain(i)
                i = nc.vector.tensor_tensor(TN["tau"][g][:], TN["tau"][g][:],
                                            TN["dlt"][g][:], ALU.add)
                dchain(i)
                i = nc.vector.tensor_scalar(TN["tauc"][g][:], TN["tau"][g][:],
                                            MAX_W, None, ALU.add)
                dchain(i)
            i = nc.vector.tensor_scalar(t1[g][:], v[g][:], TN["tau"][g][:],
                                        0.0, ALU.subtract, ALU.max)
            dchain(i)
            nc.vector.tensor_scalar(wnew[:], t1[g][:], MAX_W, None, ALU.min)
            i = nc.vector.scalar_tensor_tensor(
                dw[g][:], t1[g][:], MAX_W, wold[:], ALU.min, ALU.subtract)
            dchain(i)
            zi = nc.vector.scalar_tensor_tensor(
                z[g][:], dw[g][:], TN["th"][g][:], wnew[:], ALU.mult, ALU.add)
            E_z[g] = inc("dve", zi)
            # off the critical path: thwold for the next round's z
            i = nc.vector.tensor_scalar(thwold[g][:], wnew[:],
                                        TN["th"][g][:], None, ALU.mult)
            dchain(i)
            return
        # warm rounds: 1 Newton step; the slope count is taken at tau_old
        # within the same dchain block as the sums
        nc.vector.tensor_scalar(dum[g][:], v[g][:], TN["tau"][g][:],
                                None, ALU.max, ALU.add,
                                accum_out=TN["s1"][g][:])
        nc.vector.tensor_scalar(dum[g][:], v[g][:], TN["tauc"][g][:],
                                None, ALU.max, ALU.add,
                                accum_out=TN["s2"][g][:])
        nc.vector.tensor_scalar(dum[g][:], v[g][:], TN["tau"][g][:],
                                None, ALU.is_gt, ALU.add,
                                accum_out=TN["c1"][g][:])
        i = nc.vector.tensor_scalar(u0[g][:], v[g][:], TN["tau"][g][:],
                                    None, ALU.subtract)
        dchain(i)
        nc.vector.scalar_tensor_tensor(
            TN["phi"][g][:], TN["s1"][g][:], -(P * MAX_W - 1.0),
            TN["s2"][g][:], ALU.subtract, ALU.subtract)
        i = nc.vector.tensor_scalar(TN["cnt"][g][:], TN["c1"][g][:],
                                    1.0, None, ALU.add)
        dchain(i)
        i = nc.vector.reciprocal(TN["rc"][g][:], TN["cnt"][g][:])
        dchain(i)
        i = nc.vector.tensor_scalar(TN["dlt"][g][:], TN["phi"][g][:],
                                    TN["rc"][g][:], None, ALU.mult)
        dchain(i)
        i = nc.vector.tensor_scalar(t1[g][:], u0[g][:], TN["dlt"][g][:],
                                    0.0, ALU.subtract, ALU.max)
        dchain(i)
        if not last:
            # ws = min(t1,c)*(1+th) ; z = ws - th*w_old
            i = nc.vector.tensor_scalar(wnew[:], t1[g][:], MAX_W,
                                        TN["opth"][g][:], ALU.min, ALU.mult)
            dchain(i)
            zi = nc.vector.tensor_tensor(z[g][:], wnew[:], thwold[g][:],
                                         ALU.subtract)
            E_z[g] = inc("dve", zi)
            # off the critical path (single trailing dchain covers all):
            # tauc from tau_old + dlt (no RAW on the new tau), then tau,
            # then th*w for the next round's z
            nc.vector.scalar_tensor_tensor(
                TN["tauc"][g][:], TN["dlt"][g][:], MAX_W, TN["tau"][g][:],
                ALU.add, ALU.add)
            nc.vector.tensor_tensor(TN["tau"][g][:], TN["tau"][g][:],
                                    TN["dlt"][g][:], ALU.add)
            i = nc.vector.tensor_scalar(thwold[g][:], wnew[:],
                                        TN["thr"][g][:], None, ALU.mult)
            dchain(i)
        else:
            # renormalize and stage the output
            i = nc.vector.tensor_scalar(wnew[:], t1[g][:], MAX_W, None,
                                        ALU.min, ALU.add,
                                        accum_out=TN["ssum"][g][:])
            dchain(i)
            i = nc.vector.tensor_scalar(TN["ssum"][g][:],
                                        TN["ssum"][g][:], EPS, None,
                                        ALU.add)
            dchain(i)
            i = nc.vector.reciprocal(TN["rs"][g][:], TN["ssum"][g][:])
            dchain(i)
            oi = nc.vector.tensor_scalar(outt[g][:], wnew[:],
                                         TN["rs"][g][:], None, ALU.mult)
            E_out[g] = inc("dve", oi)

    E_ysm = [0] * NGRP
    emit_matvec(0, 0)
    for ti in range(T_FISTA):
        emit_matvec(1, ti)
        emit_chain(0, ti)
        if ti + 1 < T_FISTA:
            emit_matvec(0, ti + 1)
        emit_chain(1, ti)

    # ---------------- store
    for g in range(NGRP):
        g0 = g * GB
        wait("sync", "dve", E_out[g])
        d = nc.sync.dma_start(out=out_d[g0:g0 + GB, :], in_=outt[g][:])
        d.then_inc(sems["dma_out"], 16)
    nc.sync.wait_ge(sems["dma_out"], 16 * NGRP)


def build(lam1, lam2):
    nc = bass.Bass("TRN2", target_bir_lowering=False, debug=False)
    sigma_d = nc.dram_tensor("sigma", [NB, P, P], F16, kind="ExternalInput")
    beta_d = nc.dram_tensor("beta", [NB, P], F32, kind="ExternalInput")
    wprev_d = nc.dram_tensor("w_prev", [NB, P], F32, kind="ExternalInput")
    out_d = nc.dram_tensor("out", [NB, P], F32, kind="ExternalOutput")
    with ExitStack() as ctx:
        _emit(ctx, nc, sigma_d.ap(), beta_d.ap(), wprev_d.ap(), out_d.ap(),
              lam1, lam2)
    return nc


def kernel(sigma, beta, w_prev, log_lambda1, log_lambda2):
    global LAST_RESULT
    sigma = np.ascontiguousarray(np.asarray(sigma, dtype=np.float32))
    beta = np.ascontiguousarray(np.asarray(beta, dtype=np.float32))
    w_prev = np.ascontiguousarray(np.asarray(w_prev, dtype=np.float32))
    lam1 = float(np.exp(np.float32(log_lambda1)))
    lam2 = float(np.exp(np.float32(log_lambda2)))

    nc = build(lam1, lam2)
    in_maps = []
    for c in range(N_CORES):
        s = slice(c * NB, (c + 1) * NB)
        in_maps.append({
            "sigma": np.ascontiguousarray(sigma[s].astype(np.float16)),
            "beta": beta[s],
            "w_prev": w_prev[s],
        })
    res = run_bass_kernel_spmd(nc, in_maps, list(range(N_CORES)), trace=TRACE)
    LAST_RESULT = res
    out = np.concatenate([res.results[c]["out"] for c in range(N_CORES)],
                         axis=0)
    return np.ascontiguousarray(out.astype(np.float32))
